# revision 1
# baseline (speedup 1.0000x reference)
"""BiLSTM-CRF Viterbi decode on 8 Trainium2 NeuronCores.

Data-parallel over batch: each core handles 16 of 128 sequences.

Per-core phases:
  P0 embedding gather (indirect DMA, 128 rows per DMA)
  P1 PE-transpose x_rows [tok,E] -> x_T [E,tok] and x_Trev (time-reversed
     per sequence, via anti-diagonal identity)
  P2 bulk input projection xproj = Wih_g @ x_T (+bias) staged to DRAM,
     fp32r matmuls (N=512); bwd direction projected from x_Trev so its
     DRAM layout is s-ordered (s = T-1-t)
  P3 512 fused fwd+bwd LSTM rounds (gate-dim on partitions, fp16
     recurrence matmuls [128,16], fp32 cell state); one [128,128]
     xproj+psum add per round; ping-pong PSUM so rounds pipeline
  P4 emissions em^T = W_out @ h as [9,512] fp16 matmuls staged to DRAM
  P5 blocked max-plus scan Viterbi: time-chunks on partitions
     (p = b*8 + c, 64 steps/chunk), within-chunk prefix/suffix 9x9
     max-plus matrix products on DVE, tiny cross-chunk chains, then one
     bulk op pair recovers every alpha_t / beta_t
  P6 tags_t = argmax_i(alpha_t + beta_t), bulk [128, 64*9] ops

Precision: xproj fp32r (~2^-17), recurrence fp16 (~2^-11), DP fp32.
Measured 14-15 flipped tags of 65536 (rel err ~8e-3, gate 2e-2).
"""

import ml_dtypes
import numpy as np

import concourse.bacc as bacc
import concourse.bass as bass
import concourse.mybir as mybir
import concourse.tile as tile
from concourse.bass import IndirectOffsetOnAxis
from concourse.bass_utils import run_bass_kernel_spmd
from concourse.masks import make_identity

# Note: --enable-ldw-opt=true crashes walrus codegen (visitInstLdweights)
# on this compiler build, so the 107 ns per-128-col LDWEIGHTS stands.

F32 = mybir.dt.float32
F32R = mybir.dt.float32r
BF16 = mybir.dt.bfloat16
F16 = mybir.dt.float16
I32 = mybir.dt.int32
REC_BF16 = True       # fp16 LSTM recurrence (1-pass matmuls, FWL ldweights)
Alu = mybir.AluOpType
Act = mybir.ActivationFunctionType
AxX = mybir.AxisListType.X

B, T, V, E, H, K = 128, 512, 100000, 128, 128, 9
NC = 8
Bc = B // NC          # 16 sequences per core
TOK = Bc * T          # 8192 tokens per core, flat index bt = b*T + t (b-major)
NBLK = TOK // 128     # 64 gather/transpose blocks
G4 = 4
# gate order in weights: i, f, g, o (torch). psum cols per dir: i(0) f(16) o(32) g(48)
GOFF = {0: 0, 1: 16, 3: 32, 2: 48}
CH = 32               # LSTM rounds per xproj chunk
NCH = T // CH
DPCH = 32             # Viterbi DP steps per slab chunk
NEG = -1.0e9


def f32(ap):
    return ap.bitcast(F32)


def build_program():
    nc = bacc.Bacc(None, target_bir_lowering=False)

    # ---------------- dram parameters ----------------
    embed = nc.declare_dram_parameter("embed", [V, E], F32, isOutput=False)
    idx = nc.declare_dram_parameter("idx", [128, NBLK], I32, isOutput=False)
    RDT = F16 if REC_BF16 else F32
    whh_pack = nc.declare_dram_parameter("whh_pack", [128, 1024], RDT, isOutput=False)
    wih_pack = nc.declare_dram_parameter("wih_pack", [128, 1024], F32, isOutput=False)
    bias_pack = nc.declare_dram_parameter("bias_pack", [128, 8], F32, isOutput=False)
    lens_b = nc.declare_dram_parameter("lens_b", [128, Bc], F32, isOutput=False)
    woutT = nc.declare_dram_parameter("woutT", [128, 18], F32, isOutput=False)
    bout9 = nc.declare_dram_parameter("bout9", [K, 1], F32, isOutput=False)
    rev_id = nc.declare_dram_parameter("rev_id", [128, 128], F32, isOutput=False)
    trans81f = nc.declare_dram_parameter("trans81f", [128, 81], F32, isOutput=False)
    identM81f = nc.declare_dram_parameter("identM81f", [128, 81], F32, isOutput=False)
    maskA = nc.declare_dram_parameter("maskA", [128, 64], F32, isOutput=False)
    invmA = nc.declare_dram_parameter("invmA", [128, 64], F32, isOutput=False)
    tagm128 = nc.declare_dram_parameter("tagm128", [128, 64], F32, isOutput=False)
    irev128 = nc.declare_dram_parameter("irev128", [128, K], F32, isOutput=False)
    start9 = nc.declare_dram_parameter("start9", [Bc, K], F32, isOutput=False)
    end9 = nc.declare_dram_parameter("end9", [Bc, K], F32, isOutput=False)
    tags_out = nc.declare_dram_parameter("tags", [Bc, T], I32, isOutput=True)

    # ---------------- dram internals ----------------
    xproj_dram = nc.dram_tensor("xproj_dram", [2, G4, Bc, 128, T], F32)
    emT_dram = nc.dram_tensor("emT_dram", [K, TOK], F32)

    with tile.TileContext(nc) as tc:
        with (
            tc.tile_pool(name="big", bufs=1) as big,
            tc.tile_pool(name="consts", bufs=1) as cst,
            tc.tile_pool(name="small", bufs=4) as sm,
        ):
            # ---------- constants ----------
            idx_sb = cst.tile([128, NBLK], I32)
            nc.sync.dma_start(out=idx_sb[:], in_=idx[:])
            whh_sb = cst.tile([128, 1024], RDT)
            nc.sync.dma_start(out=whh_sb[:], in_=whh_pack[:])
            wih_sb = cst.tile([128, 1024], F32)
            nc.sync.dma_start(out=wih_sb[:], in_=wih_pack[:])
            bias_sb = cst.tile([128, 8], F32)
            nc.sync.dma_start(out=bias_sb[:], in_=bias_pack[:])
            lens_sb = cst.tile([128, Bc], F32)
            nc.sync.dma_start(out=lens_sb[:], in_=lens_b[:])
            woutT_sb = cst.tile([128, 18], F32)
            nc.sync.dma_start(out=woutT_sb[:], in_=woutT[:])
            # device-rounded fp32r copies (the fp32r matmul path requires
            # its operands to be produced pre-rounded to fp32r)
            wihr_sb = cst.tile([128, 1024], F32R)
            nc.vector.tensor_copy(out=wihr_sb[:], in_=wih_sb[:])
            woutTr_sb = cst.tile([128, 18], F16 if REC_BF16 else F32R)
            nc.vector.tensor_copy(out=woutTr_sb[:], in_=woutT_sb[:])
            bout9_sb = cst.tile([K, 1], F32)
            nc.sync.dma_start(out=bout9_sb[:], in_=bout9[:])
            rev_dma = cst.tile([128, 128], F32)
            nc.sync.dma_start(out=rev_dma[:], in_=rev_id[:])
            rev_sb = cst.tile([128, 128], F32)
            nc.vector.tensor_copy(out=rev_sb[:], in_=rev_dma[:])
            ident = cst.tile([128, 128], F32)
            make_identity(nc, ident[:])

            # DP constants (p0-15)
            start9_sb = cst.tile([Bc, K], F32)
            nc.sync.dma_start(out=start9_sb[:], in_=start9[:])
            end9_sb = cst.tile([Bc, K], F32)
            nc.sync.dma_start(out=end9_sb[:], in_=end9[:])

            # PE "absorber" ops: self-loading (fp32/fp32r) matmuls may carry
            # at most ONE sync wait in walrus codegen. These tiny ops advance
            # PE's vector clock over one-time deps (identity from Pool,
            # const-weight DMA lanes) so real matmuls each need <=1 wait.
            psp_cm = tc.tile_pool(name="psglob", bufs=1, space="PSUM")
            psp = psp_cm.__enter__()
            pq1 = psp.tile([128, 512], F32, tag="pq1", name="pq1")
            pq2 = psp.tile([128, 512], F32, tag="pq2", name="pq2")
            pw1 = psp.tile([128, 512], F32, tag="pw1", name="pw1")
            pw2 = psp.tile([128, 512], F32, tag="pw2", name="pw2")
            pw3 = psp.tile([128, 512], F32, tag="pw3", name="pw3")
            nc.tensor.transpose(out=pq1[:, 0:128], in_=ident[:], identity=ident[:])
            nc.tensor.transpose(out=pq2[:, 0:128], in_=rev_sb[:], identity=ident[:])
            nc.tensor.matmul(out=pq2[0:1, 0:1], lhsT=whh_sb[:, 0:1],
                             rhs=whh_sb[:, 0:1], start=True, stop=True)

            # ---------- P0: gather ----------
            x_T = big.tile([128, TOK], F32R, tag="bigA")
            x_Trev = big.tile([128, TOK], F32R, tag="bigB")
            with tc.tile_pool(name="xr", bufs=24) as xrp:
                x_rows = []
                for g in range(NBLK):
                    xr = xrp.tile([128, 128], F32, tag="xr")
                    nc.gpsimd.indirect_dma_start(
                        out=xr[:],
                        out_offset=None,
                        in_=embed[:],
                        in_offset=IndirectOffsetOnAxis(
                            ap=idx_sb[:, g:g + 1], axis=0),
                    )
                    x_rows.append(xr)

                # ---------- P1: transpose (fwd + time-reversed) ----------
                with tc.tile_pool(name="xrel", bufs=4) as xrelp:
                    psts = [pq1, pq2]
                    for g in range(NBLK):
                        b_seq, tb4 = g // 4, g % 4
                        grev = b_seq * 4 + (3 - tb4)
                        xrel = xrelp.tile([128, 128], F32, tag="xrel")
                        nc.vector.tensor_tensor(
                            out=xrel[:], in0=x_rows[g][:], in1=x_rows[g][:],
                            op=Alu.max)
                        pst = psts[g % 2]
                        nc.tensor.transpose(
                            out=pst[:, 0:128], in_=xrel[:], identity=ident[:])
                        nc.vector.tensor_copy(
                            out=x_T[:, g * 128:(g + 1) * 128],
                            in_=pst[:, 0:128])
                        nc.tensor.transpose(
                            out=pst[:, 128:256], in_=xrel[:],
                            identity=rev_sb[:])
                        nc.vector.tensor_copy(
                            out=x_Trev[:, grev * 128:(grev + 1) * 128],
                            in_=pst[:, 128:256])

            # ---------- P2: bulk xproj (fp32r, N=512) ----------
            ps2s = [pw1[:], pw2[:], pw3[:]]
            n2 = 0
            for d in range(2):
                src_T = x_T if d == 0 else x_Trev
                for g in range(G4):
                    lhsT = wihr_sb[:, (d * G4 + g) * 128:(d * G4 + g + 1) * 128]
                    for b in range(Bc):
                        ps2 = ps2s[n2 % 3]
                        n2 += 1
                        nc.tensor.matmul(
                            out=ps2, lhsT=lhsT,
                            rhs=src_T[:, b * T:(b + 1) * T],
                            start=True, stop=True)
                        xp_sb = sm.tile([128, 512], F32, tag="xp_out")
                        nc.vector.tensor_scalar(
                            out=xp_sb[:], in0=ps2,
                            scalar1=bias_sb[:, d * G4 + g:d * G4 + g + 1],
                            scalar2=None, op0=Alu.add)
                        # store at PSUM block position (i,f,o,g order)
                        nc.sync.dma_start(
                            out=xproj_dram[d, GOFF[g] // 16, b],
                            in_=xp_sb[:])

            # ---------- P3: LSTM ----------
            HDT = F16 if REC_BF16 else F32R
            h_f = big.tile([128, TOK], HDT, tag="bigA")
            h_b = big.tile([128, TOK], HDT, tag="bigB")
            h0 = cst.tile([128, Bc], HDT)
            nc.vector.memset(h0[:], 0.0)
            c_st = cst.tile([128, 2 * Bc], F32)
            nc.vector.memset(c_st[:], 0.0)

            with tc.tile_pool(name="xpp", bufs=2) as xpp:
                for r in range(T):
                    tf, tb = r, T - 1 - r
                    c = r // CH
                    if r % CH == 0:
                        # one fused chunk: [128, (s, d, gslot, b)] — both
                        # dirs share the s index (bwd xproj is s-ordered)
                        # layout [128, (d, gslot, b, s)]: matches psum column
                        # order (d,g,b) when sliced at fixed s
                        xt = xpp.tile([128, CH * 128], F32, tag="xpc")
                        for d in range(2):
                            dst = xt[:].rearrange(
                                "p (y s) -> p y s",
                                s=CH)[:, d * 64:(d + 1) * 64]
                            src = xproj_dram[:].rearrange(
                                "e g b p t -> p (e g b) t")[
                                :, d * 64:(d + 1) * 64,
                                c * CH:(c + 1) * CH]
                            nc.sync.dma_start(out=dst, in_=src)

                    ps3 = [pq1, pq2, pw3][r % 3]  # 3-deep: decouple from gsb
                    for d in range(2):
                        if r == 0:
                            hprev = h0[:]
                        elif d == 0:
                            hprev = h_f[:, tf - 1::T]
                        else:
                            hprev = h_b[:, tb + 1::T]
                        if not REC_BF16:
                            hprev = f32(hprev)
                        for g in range(G4):
                            lhsT = whh_sb[
                                :, (d * G4 + g) * 128:(d * G4 + g + 1) * 128]
                            nc.tensor.matmul(
                                out=ps3[:, d * 64 + GOFF[g]:
                                        d * 64 + GOFF[g] + Bc],
                                lhsT=lhsT, rhs=hprev, start=True, stop=True)
                    gsb = sm.tile([128, 128], F32, tag="gates")
                    xsl = xt[:].rearrange(
                        "p (y s) -> p y s", s=CH)[:, :, r % CH]
                    nc.vector.tensor_tensor(
                        out=gsb[:], in0=ps3[:, 0:128], in1=xsl, op=Alu.add)
                    sig = sm.tile([128, 96], F32, tag="sig")
                    nc.scalar.activation(
                        out=sig[:].rearrange(
                            "p (q d b) -> p d q b", q=3, d=2, b=Bc),
                        in_=gsb[:].rearrange("p (d x) -> p d x", d=2)[:, :, 0:48],
                        func=Act.Sigmoid)
                    tg = sm.tile([128, 2 * Bc], F32, tag="tg")
                    nc.scalar.activation(
                        out=tg[:],
                        in_=gsb[:].rearrange("p (d x) -> p d x", d=2)[:, :, 48:64],
                        func=Act.Tanh)
                    t1 = sm.tile([128, 2 * Bc], F32, tag="t1")
                    nc.gpsimd.tensor_tensor(
                        out=t1[:], in0=sig[:, 0:2 * Bc], in1=tg[:], op=Alu.mult)
                    t2 = sm.tile([128, 2 * Bc], F32, tag="t2")
                    nc.vector.tensor_tensor(
                        out=t2[:], in0=sig[:, 2 * Bc:4 * Bc], in1=c_st[:],
                        op=Alu.mult)
                    nc.vector.tensor_tensor(
                        out=c_st[:], in0=t1[:], in1=t2[:], op=Alu.add)
                    tcx = sm.tile([128, 2 * Bc], F32, tag="tc")
                    nc.scalar.activation(out=tcx[:], in_=c_st[:], func=Act.Tanh)
                    mt = sm.tile([128, Bc], F32, tag="mt")
                    nc.gpsimd.tensor_scalar(
                        out=mt[:], in0=lens_sb[:], scalar1=float(tb),
                        scalar2=None, op0=Alu.is_gt)
                    nc.vector.tensor_tensor(
                        out=h_f[:, tf::T], in0=sig[:, 4 * Bc:5 * Bc],
                        in1=tcx[:, 0:Bc], op=Alu.mult)
                    hbt = sm.tile([128, Bc], F32, tag="hbt")
                    nc.gpsimd.tensor_tensor(
                        out=hbt[:], in0=sig[:, 5 * Bc:6 * Bc],
                        in1=tcx[:, Bc:2 * Bc], op=Alu.mult)
                    nc.vector.tensor_tensor(
                        out=h_b[:, tb::T], in0=hbt[:], in1=mt[:], op=Alu.mult)
                    nc.gpsimd.tensor_tensor(
                        out=c_st[:, Bc:2 * Bc], in0=c_st[:, Bc:2 * Bc],
                        in1=mt[:], op=Alu.mult)

            # ---------- P4: emissions em^T = [9, TOK] (fp32r, N=512) ----------
            for blk in range(Bc):
                ps4 = [pw1, pw2][blk % 2][0:K, :]
                sl = slice(blk * T, (blk + 1) * T)
                nc.tensor.matmul(
                    out=ps4, lhsT=woutTr_sb[:, 0:K], rhs=h_f[:, sl],
                    start=True, stop=False)
                nc.tensor.matmul(
                    out=ps4, lhsT=woutTr_sb[:, K:2 * K], rhs=h_b[:, sl],
                    start=False, stop=True)
                em_sb = sm.tile([K, T], F32, tag="em_sb")
                nc.vector.tensor_scalar(
                    out=em_sb[:], in0=ps4, scalar1=bout9_sb[:, 0:1],
                    scalar2=None, op0=Alu.add)
                nc.sync.dma_start(out=emT_dram[:, sl], in_=em_sb[:])

            # ---------- P5: blocked max-plus scan Viterbi ----------
            # partition p = b*8 + c: sequence b, time-chunk c (C=64 steps).
            # A_t[i,j] = mask_t ? trans[i,j] + em_t[j] : maxplus identity
            # (0 diag / -1e9 off); A_0 := I via maskA[.,0]=0 host fudge.
            # alpha_t = alpha_{t-1} (x) A_t  (row-vec max-plus)
            # beta_t  = A_{t+1} (x) beta_{t+1}  (col-vec max-plus)
            # tags_t  = argmax_i(alpha_t[i] + beta_t[i])
            C64, NC8 = 64, 8
            dpbig_cm = tc.tile_pool(name="dpbig", bufs=1)
            dbig = dpbig_cm.__enter__()

            em128 = dbig.tile([128, K * C64], F32, tag="em128")  # (j, k)
            nc.sync.dma_start(
                out=em128[:].rearrange("p (j k) -> p j k", j=K),
                in_=emT_dram[:].rearrange(
                    "k (b c t) -> (b c) k t", b=Bc, c=NC8))
            em0_sb = dbig.tile([Bc, K], F32, tag="em0")
            nc.sync.dma_start(
                out=em0_sb[:].unsqueeze(2),
                in_=emT_dram[:].rearrange("k (b t) -> b k t", b=Bc)[:, :, 0:1])

            trans128_sb = dbig.tile([128, 81], F32, tag="t128")
            nc.sync.dma_start(out=trans128_sb[:], in_=trans81f[:])
            identM128_sb = dbig.tile([128, 81], F32, tag="i128")
            nc.sync.dma_start(out=identM128_sb[:], in_=identM81f[:])
            maskA_sb = dbig.tile([128, C64], F32, tag="mA")
            nc.sync.dma_start(out=maskA_sb[:], in_=maskA[:])
            invmA_sb = dbig.tile([128, C64], F32, tag="imA")
            nc.sync.dma_start(out=invmA_sb[:], in_=invmA[:])
            tagm_sb = dbig.tile([128, C64], F32, tag="tagm")
            nc.sync.dma_start(out=tagm_sb[:], in_=tagm128[:])
            irev128_sb = dbig.tile([128, K], F32, tag="irev128")
            nc.sync.dma_start(out=irev128_sb[:], in_=irev128[:])

            # A slab build: A = m*(trans+em) + invm*I  (exact-zero masking)
            Aslab = dbig.tile([128, C64 * 81], F32, tag="Aslab")
            Atmp = dbig.tile([128, C64 * 81], F32, tag="Atmp")

            def A4(t_):
                return t_[:].rearrange("p (s i j) -> p s i j", s=C64, i=K)

            emv = em128[:].rearrange("p (j k) -> p k j", j=K)
            emv = emv.unsqueeze(2).to_broadcast([128, C64, K, K])
            t128v = trans128_sb[:].rearrange("p (i j) -> p i j", i=K)
            t128v = t128v.unsqueeze(1).to_broadcast([128, C64, K, K])
            i128v = identM128_sb[:].rearrange("p (i j) -> p i j", i=K)
            i128v = i128v.unsqueeze(1).to_broadcast([128, C64, K, K])
            mAv = maskA_sb[:].unsqueeze(2).unsqueeze(3).to_broadcast(
                [128, C64, K, K])
            imAv = invmA_sb[:].unsqueeze(2).unsqueeze(3).to_broadcast(
                [128, C64, K, K])
            nc.vector.tensor_tensor(out=A4(Atmp), in0=emv, in1=t128v,
                                    op=Alu.add)
            nc.vector.tensor_tensor(out=A4(Atmp), in0=A4(Atmp), in1=mAv,
                                    op=Alu.mult)
            nc.vector.tensor_tensor(out=A4(Aslab), in0=i128v, in1=imAv,
                                    op=Alu.mult)
            nc.vector.tensor_tensor(out=A4(Aslab), in0=A4(Aslab),
                                    in1=A4(Atmp), op=Alu.add)

            # transposed A copy: ATslab[s][(j,m)] = A_s[m,j] — makes every
            # level-1 operand innermost-contiguous (strided adds are ~1.7x)
            ATslab = dbig.tile([128, C64 * 81], F32, tag="ATslab")
            nc.vector.tensor_copy(
                out=ATslab[:].rearrange("p (s j m) -> p s j m", s=C64, j=K),
                in_=A4(Aslab).transpose([0, 1, 3, 2]))

            # level 1: within-chunk prefix (Pslab) and suffix (SufT) products
            # Pslab[k] = A_0..A_k (i,j); SufT[k] = (A_k..A_63)^T (j,m)
            Pslab = dbig.tile([128, C64 * 81], F32, tag="Atmp")
            SufT = dbig.tile([128, (C64 + 1) * 81], F32, tag="Suf")
            candf = dbig.tile([128, 729], F32, tag="candf")
            candb = dbig.tile([128, 729], F32, tag="candb")
            nc.vector.tensor_copy(out=SufT[:, C64 * 81:], in_=identM128_sb[:])
            nc.vector.tensor_copy(out=Pslab[:, 0:81], in_=Aslab[:, 0:81])

            def m81(ap, kk):
                return ap[:, kk * 81:(kk + 1) * 81]

            def as_ij(ap3):
                return ap3.rearrange("p (i j) -> p i j", i=K)

            def jm(ap81):
                return ap81.rearrange("p (j m) -> p j m", j=K)

            def sufT_out(kk):
                # write (i,j)-indexed result into (j,i) storage
                return m81(SufT, kk).rearrange(
                    "p (j i) -> p j i", j=K).transpose([0, 2, 1])

            def bwd_step(kb):
                av2 = as_ij(m81(Aslab, kb)).unsqueeze(2).to_broadcast(
                    [128, K, K, K])                          # (i, j, m)
                sv = jm(m81(SufT, kb + 1)).unsqueeze(1).to_broadcast(
                    [128, K, K, K])                          # (i, j, m)
                nc.vector.tensor_tensor(
                    out=candb[:].rearrange("p (i j m) -> p i j m", i=K, j=K),
                    in0=av2, in1=sv, op=Alu.add)
                nc.vector.tensor_reduce(
                    out=sufT_out(kb),
                    in_=candb[:].rearrange("p (i j m) -> p i j m", i=K, j=K),
                    axis=AxX, op=Alu.max)

            for k in range(1, C64):
                # interleave fwd step k and bwd step (C64-k) on DVE
                pv = as_ij(m81(Pslab, k - 1)).unsqueeze(2).to_broadcast(
                    [128, K, K, K])                          # (i, j, m)
                av = jm(m81(ATslab, k)).unsqueeze(1).to_broadcast(
                    [128, K, K, K])                          # (i, j, m)
                nc.vector.tensor_tensor(
                    out=candf[:].rearrange("p (i j m) -> p i j m", i=K, j=K),
                    in0=pv, in1=av, op=Alu.add)
                nc.vector.tensor_reduce(
                    out=as_ij(m81(Pslab, k)),
                    in_=candf[:].rearrange("p (i j m) -> p i j m", i=K, j=K),
                    axis=AxX, op=Alu.max)
                bwd_step(C64 - k)
            bwd_step(0)

            # level 2: cross-chunk alpha-start / beta-end chains on p0-15
            Gg = dbig.tile([Bc, NC8 * 81], F32, tag="Gg")
            nc.sync.dma_start(out=Gg[:], in_=m81(Pslab, C64 - 1))
            Sg = dbig.tile([Bc, NC8 * 81], F32, tag="Sg")
            nc.sync.dma_start(out=Sg[:], in_=m81(SufT, 0))
            ast = dbig.tile([Bc, NC8 * K], F32, tag="ast")
            bend = dbig.tile([Bc, NC8 * K], F32, tag="bend")
            candL = dbig.tile([Bc, 81], F32, tag="candL")
            nc.vector.tensor_tensor(
                out=ast[:, 0:K], in0=start9_sb[:], in1=em0_sb[:], op=Alu.add)
            for c in range(1, NC8):
                # ast_c[j] = max_i ast_{c-1}[i] + G_{c-1}[i,j]
                in0 = ast[:, (c - 1) * K:c * K].unsqueeze(1).to_broadcast(
                    [Bc, K, K])                               # (j, i)
                in1 = Gg[:, (c - 1) * 81:c * 81].rearrange(
                    "b (i j) -> b i j", i=K).transpose([0, 2, 1])
                nc.vector.tensor_tensor(
                    out=candL[:].rearrange("b (j i) -> b j i", j=K),
                    in0=in0, in1=in1, op=Alu.add)
                nc.vector.tensor_reduce(
                    out=ast[:, c * K:(c + 1) * K],
                    in_=candL[:].rearrange("b (j i) -> b j i", j=K),
                    axis=AxX, op=Alu.max)
            nc.vector.tensor_copy(
                out=bend[:, (NC8 - 1) * K:], in_=end9_sb[:])
            for c in range(NC8 - 2, -1, -1):
                # bend_c[i] = max_j S0_{c+1}[i,j] + bend_{c+1}[j]
                in0 = bend[:, (c + 1) * K:(c + 2) * K].unsqueeze(1) \
                    .to_broadcast([Bc, K, K])                 # (i, j)
                in1 = Sg[:, (c + 1) * 81:(c + 2) * 81].rearrange(
                    "b (j i) -> b j i", j=K).transpose([0, 2, 1])
                nc.vector.tensor_tensor(
                    out=candL[:].rearrange("b (i j) -> b i j", i=K),
                    in0=in0, in1=in1, op=Alu.add)
                nc.vector.tensor_reduce(
                    out=bend[:, c * K:(c + 1) * K],
                    in_=candL[:].rearrange("b (i j) -> b i j", i=K),
                    axis=AxX, op=Alu.max)
            alpha_start = dbig.tile([128, K], F32, tag="astart")
            nc.sync.dma_start(out=alpha_start[:], in_=ast[:])
            beta_end = dbig.tile([128, K], F32, tag="bstart")
            nc.sync.dma_start(out=beta_end[:], in_=bend[:])

            # level 3: all per-step alpha/beta via one bulk op pair each
            cand3 = dbig.tile([128, C64 * 81], F32, tag="Aslab")
            alpha_all = dbig.tile([128, C64 * K], F32, tag="em128")
            in0 = alpha_start[:].unsqueeze(1).unsqueeze(1).to_broadcast(
                [128, C64, K, K])                             # (k, j, i)
            in1 = Pslab[:].rearrange(
                "p (k i j) -> p k i j", k=C64, i=K).transpose([0, 1, 3, 2])
            nc.vector.tensor_tensor(
                out=cand3[:].rearrange("p (k j i) -> p k j i", k=C64, j=K),
                in0=in0, in1=in1, op=Alu.add)
            nc.vector.tensor_reduce(
                out=alpha_all[:].rearrange("p (k j) -> p k j", k=C64),
                in_=cand3[:].rearrange("p (k j i) -> p k j i", k=C64, j=K),
                axis=AxX, op=Alu.max)
            cand4 = dbig.tile([128, C64 * 81], F32, tag="Atmp")
            beta_all = dbig.tile([128, C64 * K], F32, tag="ball")
            in0 = beta_end[:].unsqueeze(1).unsqueeze(1).to_broadcast(
                [128, C64, K, K])                             # (k, i, j)
            in1 = SufT[:, 81:].rearrange(
                "p (k j i) -> p k j i", k=C64, j=K).transpose([0, 1, 3, 2])
            nc.vector.tensor_tensor(
                out=cand4[:].rearrange("p (k i j) -> p k i j", k=C64, i=K),
                in0=in0, in1=in1, op=Alu.add)
            nc.vector.tensor_reduce(
                out=beta_all[:].rearrange("p (k i) -> p k i", k=C64),
                in_=cand4[:].rearrange("p (k i j) -> p k i j", k=C64, i=K),
                axis=AxX, op=Alu.max)

            # ---------- P6: tags = argmax_i(alpha + beta), mask, emit ------
            nc.vector.tensor_tensor(
                out=alpha_all[:], in0=alpha_all[:], in1=beta_all[:],
                op=Alu.add)
            mx128 = dbig.tile([128, C64], F32, tag="mx128")
            nc.vector.tensor_reduce(
                out=mx128[:],
                in_=alpha_all[:].rearrange("p (k j) -> p k j", k=C64),
                axis=AxX, op=Alu.max)
            nc.vector.tensor_tensor(
                out=beta_all[:].rearrange("p (k j) -> p k j", k=C64),
                in0=alpha_all[:].rearrange("p (k j) -> p k j", k=C64),
                in1=mx128[:].unsqueeze(2).to_broadcast([128, C64, K]),
                op=Alu.is_equal)
            nc.vector.tensor_tensor(
                out=beta_all[:].rearrange("p (k j) -> p k j", k=C64),
                in0=beta_all[:].rearrange("p (k j) -> p k j", k=C64),
                in1=irev128_sb[:].unsqueeze(1).to_broadcast([128, C64, K]),
                op=Alu.mult)
            tags128 = dbig.tile([128, C64], F32, tag="tags128")
            nc.vector.tensor_reduce(
                out=tags128[:],
                in_=beta_all[:].rearrange("p (k j) -> p k j", k=C64),
                axis=AxX, op=Alu.max)
            nc.vector.tensor_scalar(
                out=tags128[:], in0=tags128[:], scalar1=-1.0, scalar2=8.0,
                op0=Alu.mult, op1=Alu.add)
            nc.vector.tensor_tensor(
                out=tags128[:], in0=tags128[:], in1=tagm_sb[:], op=Alu.mult)
            tagsi = dbig.tile([128, C64], I32, tag="tagsi")
            nc.vector.tensor_copy(out=tagsi[:], in_=tags128[:])
            nc.sync.dma_start(out=tags_out[:], in_=tagsi[:])
            dpbig_cm.__exit__(None, None, None)
            psp_cm.__exit__(None, None, None)

    nc.finalize()
    return nc


_NC_CACHE = None


def _get_program():
    global _NC_CACHE
    if _NC_CACHE is None:
        _NC_CACHE = build_program()
    return _NC_CACHE


def make_in_maps(sentences, lengths, embed, Wih_f, Whh_f, bih_f, bhh_f,
                 Wih_b, Whh_b, bih_b, bhh_b, W_out, b_out, start_t, end_t,
                 trans):
    sentences = np.ascontiguousarray(sentences, dtype=np.int32)
    embed = np.ascontiguousarray(embed, dtype=np.float32)
    lengths = np.asarray(lengths)

    whh_pack = np.zeros((128, 1024), np.float32)
    wih_pack = np.zeros((128, 1024), np.float32)
    bias_pack = np.zeros((128, 8), np.float32)
    for d, (Wih, Whh, bi, bh) in enumerate(
            ((Wih_f, Whh_f, bih_f, bhh_f), (Wih_b, Whh_b, bih_b, bhh_b))):
        for g in range(G4):
            whh_pack[:, (d * G4 + g) * 128:(d * G4 + g + 1) * 128] = \
                np.asarray(Whh)[g * 128:(g + 1) * 128, :].T
            wih_pack[:, (d * G4 + g) * 128:(d * G4 + g + 1) * 128] = \
                np.asarray(Wih)[g * 128:(g + 1) * 128, :].T
            bias_pack[:, d * G4 + g] = \
                (np.asarray(bi) + np.asarray(bh))[g * 128:(g + 1) * 128]

    W_out = np.asarray(W_out, np.float32)
    woutT = np.zeros((128, 18), np.float32)
    woutT[:, 0:K] = W_out[:, :128].T
    woutT[:, K:2 * K] = W_out[:, 128:].T
    bout9 = np.asarray(b_out, np.float32)[:, None].copy()

    rev_id = np.zeros((128, 128), np.float32)
    rev_id[np.arange(128), 127 - np.arange(128)] = 1.0

    trans_np = np.asarray(trans, np.float32)
    identM = np.full((K, K), NEG, np.float32)
    np.fill_diagonal(identM, 0.0)
    trans81f = np.broadcast_to(trans_np.reshape(-1)[None], (128, 81)).copy()
    identM81f = np.broadcast_to(identM.reshape(-1)[None], (128, 81)).copy()

    start9 = np.broadcast_to(
        np.asarray(start_t, np.float32)[None, :], (Bc, K)).copy()
    end9 = np.broadcast_to(
        np.asarray(end_t, np.float32)[None, :], (Bc, K)).copy()
    ii = np.arange(K, dtype=np.float32)
    irev128 = np.broadcast_to((8.0 - ii)[None, :], (128, K)).copy()
    tt = np.arange(T)

    in_maps = []
    for c in range(NC):
        sl = slice(c * Bc, (c + 1) * Bc)
        sents_c = sentences[sl]
        lens_c = np.asarray(lengths[sl], np.float32)
        idx_np = np.zeros((128, NBLK), np.int32)
        p = np.arange(128)
        for g in range(NBLK):
            bt = g * 128 + p
            idx_np[:, g] = sents_c[bt // T, bt % T]
        lens_bc = np.broadcast_to(lens_c[None, :], (128, Bc)).copy()
        mask_np = (tt[None, :] < lens_c[:, None]).astype(np.float32)
        # [128, 64] chunked masks, partition p = b*8 + c
        tagm = mask_np.reshape(Bc * 8, 64).copy()
        maskA_np = tagm.copy()
        maskA_np[0::8, 0] = 0.0  # A_0 := maxplus identity
        whh_send = whh_pack.astype(np.float16) if REC_BF16 else whh_pack
        in_maps.append({
            "embed": embed,
            "idx": idx_np,
            "whh_pack": whh_send, "wih_pack": wih_pack, "bias_pack": bias_pack,
            "lens_b": lens_bc,
            "woutT": woutT, "bout9": bout9, "rev_id": rev_id,
            "trans81f": trans81f, "identM81f": identM81f,
            "maskA": maskA_np, "invmA": 1.0 - maskA_np, "tagm128": tagm,
            "irev128": irev128,
            "start9": start9, "end9": end9,
        })
    return in_maps


def run(inputs, trace=False, **kw):
    nc = _get_program()
    in_maps = make_in_maps(**inputs)
    res = run_bass_kernel_spmd(nc, in_maps, list(range(NC)), trace=trace, **kw)
    tags = np.concatenate([r["tags"] for r in res.results], axis=0)
    return tags.astype(np.int32), res


def kernel(**inputs):
    tags, _ = run(inputs)
    return tags



# revision 12
# speedup vs baseline: 1.2760x; 1.2760x over previous
"""BiLSTM-CRF Viterbi decode on 8 Trainium2 NeuronCores.

Data-parallel over batch: each core handles 16 of 128 sequences.

Per-core phases:
  P0 embedding gather (indirect DMA, 128 rows per DMA)
  P1 PE-transpose x_rows [tok,E] -> x_T [E,tok] and x_Trev (time-reversed
     per sequence, via anti-diagonal identity)
  P2 bulk input projection xproj = Wih_g @ x_T (+bias) staged to DRAM fp16,
     fp32r matmuls (N=512); bwd direction projected from x_Trev so its
     DRAM layout is s-ordered (s = T-1-t). Bwd i/f gate lanes at padded
     steps are poisoned to -1e4 so tanh saturates to -1 and the cell
     state provably stays zero through the padded prefix (replaces all
     per-round masking).
  P3 512 fused fwd+bwd LSTM rounds, all-tanh formulation:
     host pre-scales weights so ONE tanh over the [128,128] gate PSUM
     yields s = tanh(x/2) for i,f,o (i.e. 2*sigmoid-1) and tanh(g) for g.
     Cell state kept as C = 2c, hidden as H = 2h:
        t1 = (s_i+1)*tg          (= 2 sig_i tg)
        t2 = (s_f+1)*C           (= 2 sig_f C)
        C' = t2*0.5 + t1         (= 2c')
        tcx = tanh(0.5*C')       (= tanh(c'))
        H' = (s_o+1)*tcx         (= 2h')
     xproj joins the gates via an fp16 identity matmul accumulating into
     the same PSUM bank (no DVE add). Both directions' H live in one
     [128,32] slot per round (bwd s-ordered) so one STT writes both and
     all 8 recurrence matmuls wait on a single semaphore.
  P4 emissions em^T = W_out @ h as [9,512] fp16 matmuls (strided /
     reverse-strided rhs straight out of h_all) staged to DRAM
  P5 blocked max-plus scan Viterbi: time-chunks on partitions
     (p = b*8 + c, 64 steps/chunk), within-chunk prefix/suffix 9x9
     max-plus matrix products on DVE, tiny cross-chunk chains, then one
     bulk op pair recovers every alpha_t / beta_t
  P6 tags_t = argmax_i(alpha_t + beta_t), bulk [128, 64*9] ops

Precision: xproj fp32r matmul staged fp16, recurrence fp16, DP fp32.
"""

import ml_dtypes
import numpy as np

import concourse.bacc as bacc
import concourse.bass as bass
import concourse.mybir as mybir
import concourse.tile as tile
from concourse.bass import IndirectOffsetOnAxis
from concourse.bass_utils import run_bass_kernel_spmd
from concourse.masks import make_identity

F32 = mybir.dt.float32
F32R = mybir.dt.float32r
BF16 = mybir.dt.bfloat16
F16 = mybir.dt.float16
I32 = mybir.dt.int32
Alu = mybir.AluOpType
Act = mybir.ActivationFunctionType
AxX = mybir.AxisListType.X

B, T, V, E, H, K = 128, 512, 100000, 128, 128, 9
NC = 8
Bc = B // NC          # 16 sequences per core
TOK = Bc * T          # 8192 tokens per core, flat index bt = b*T + t (b-major)
NBLK = TOK // 128     # 64 gather/transpose blocks
G4 = 4
# gate order in weights: i, f, g, o (torch). psum cols per dir: i(0) f(16) o(32) g(48)
GOFF = {0: 0, 1: 16, 3: 32, 2: 48}
CH = 32               # LSTM rounds per xproj chunk
NCH = T // CH
NEG = -1.0e9
POIS = -1.0e4         # bwd i/f gate poison at padded steps
XDT = F16             # xproj staging dtype


def f32(ap):
    return ap.bitcast(F32)


def build_program():
    nc = bacc.Bacc(None, target_bir_lowering=False)

    # ---------------- dram parameters ----------------
    embed = nc.declare_dram_parameter("embed", [V, E], F32, isOutput=False)
    idx = nc.declare_dram_parameter("idx", [128, NBLK], I32, isOutput=False)
    whh_pack = nc.declare_dram_parameter("whh_pack", [128, 1024], F16, isOutput=False)
    wih_pack = nc.declare_dram_parameter("wih_pack", [128, 1024], F32, isOutput=False)
    bias_pack = nc.declare_dram_parameter("bias_pack", [128, 8], F32, isOutput=False)
    mpois = nc.declare_dram_parameter("mpois", [128, TOK], F16, isOutput=False)
    woutT = nc.declare_dram_parameter("woutT", [128, 18], F32, isOutput=False)
    bout9 = nc.declare_dram_parameter("bout9", [K, 1], F32, isOutput=False)
    rev_id = nc.declare_dram_parameter("rev_id", [128, 128], F32, isOutput=False)
    trans81f = nc.declare_dram_parameter("trans81f", [128, 81], F32, isOutput=False)
    identM81f = nc.declare_dram_parameter("identM81f", [128, 81], F32, isOutput=False)
    maskA = nc.declare_dram_parameter("maskA", [128, 64], F32, isOutput=False)
    invmA = nc.declare_dram_parameter("invmA", [128, 64], F32, isOutput=False)
    tagm128 = nc.declare_dram_parameter("tagm128", [128, 64], F32, isOutput=False)
    irev128 = nc.declare_dram_parameter("irev128", [128, K], F32, isOutput=False)
    start9 = nc.declare_dram_parameter("start9", [Bc, K], F32, isOutput=False)
    end9 = nc.declare_dram_parameter("end9", [Bc, K], F32, isOutput=False)
    tags_out = nc.declare_dram_parameter("tags", [Bc, T], I32, isOutput=True)

    # ---------------- dram internals ----------------
    xproj_dram = nc.dram_tensor("xproj_dram", [2, G4, Bc, 128, T], XDT)
    emT_dram = nc.dram_tensor("emT_dram", [K, TOK], F32)

    with tile.TileContext(nc) as tc:
        with (
            tc.tile_pool(name="big", bufs=1) as big,
            tc.tile_pool(name="consts", bufs=1) as cst,
            tc.tile_pool(name="small", bufs=4) as sm,
        ):
            # ---------- constants ----------
            idx_sb = cst.tile([128, NBLK], I32)
            nc.sync.dma_start(out=idx_sb[:], in_=idx[:])
            whh_sb = cst.tile([128, 1024], F16)
            nc.sync.dma_start(out=whh_sb[:], in_=whh_pack[:])
            wih_sb = cst.tile([128, 1024], F32)
            nc.sync.dma_start(out=wih_sb[:], in_=wih_pack[:])
            bias_sb = cst.tile([128, 8], F32)
            nc.sync.dma_start(out=bias_sb[:], in_=bias_pack[:])
            mpp_cm = tc.tile_pool(name="mpp", bufs=1)
            mpp = mpp_cm.__enter__()
            mpois_sb = mpp.tile([128, TOK], F16)
            nc.sync.dma_start(out=mpois_sb[:], in_=mpois[:])
            woutT_sb = cst.tile([128, 18], F32)
            nc.sync.dma_start(out=woutT_sb[:], in_=woutT[:])
            # device-rounded fp32r copies (the fp32r matmul path requires
            # its operands to be produced pre-rounded to fp32r)
            wihr_sb = cst.tile([128, 1024], F32R)
            nc.vector.tensor_copy(out=wihr_sb[:], in_=wih_sb[:])
            woutTr_sb = cst.tile([128, 18], F16)
            nc.vector.tensor_copy(out=woutTr_sb[:], in_=woutT_sb[:])
            bout9_sb = cst.tile([K, 1], F32)
            nc.sync.dma_start(out=bout9_sb[:], in_=bout9[:])
            rev_dma = cst.tile([128, 128], F32)
            nc.sync.dma_start(out=rev_dma[:], in_=rev_id[:])
            rev_sb = cst.tile([128, 128], F32)
            nc.vector.tensor_copy(out=rev_sb[:], in_=rev_dma[:])
            ident = cst.tile([128, 128], F32)
            make_identity(nc, ident[:])
            ident16 = cst.tile([128, 128], F16)
            nc.vector.tensor_copy(out=ident16[:], in_=ident[:])

            # DP constants (p0-15)
            start9_sb = cst.tile([Bc, K], F32)
            nc.sync.dma_start(out=start9_sb[:], in_=start9[:])
            end9_sb = cst.tile([Bc, K], F32)
            nc.sync.dma_start(out=end9_sb[:], in_=end9[:])

            # PE "absorber" ops: self-loading (fp32/fp32r) matmuls may carry
            # at most ONE sync wait in walrus codegen. These tiny ops advance
            # PE's vector clock over one-time deps (identity from Pool,
            # const-weight DMA lanes) so real matmuls each need <=1 wait.
            psp_cm = tc.tile_pool(name="psglob", bufs=1, space="PSUM")
            psp = psp_cm.__enter__()
            pq1 = psp.tile([128, 512], F32, tag="pq1", name="pq1")
            pq2 = psp.tile([128, 512], F32, tag="pq2", name="pq2")
            pw1 = psp.tile([128, 512], F32, tag="pw1", name="pw1")
            pw2 = psp.tile([128, 512], F32, tag="pw2", name="pw2")
            pw3 = psp.tile([128, 512], F32, tag="pw3", name="pw3")
            nc.tensor.transpose(out=pq1[:, 0:128], in_=ident[:], identity=ident[:])
            nc.tensor.transpose(out=pq2[:, 0:128], in_=rev_sb[:], identity=ident[:])
            nc.tensor.matmul(out=pq2[0:1, 0:1], lhsT=whh_sb[:, 0:1],
                             rhs=whh_sb[:, 0:1], start=True, stop=True)
            nc.tensor.matmul(out=pq1[0:1, 0:1], lhsT=ident16[:, 0:1],
                             rhs=whh_sb[:, 0:1], start=True, stop=True)

            # ---------- P0: gather ----------
            x_T = big.tile([128, TOK], F32R, tag="bigA")
            x_Trev = big.tile([128, TOK], F32R, tag="bigB")
            with tc.tile_pool(name="xr", bufs=24) as xrp:
                x_rows = []
                for g in range(NBLK):
                    xr = xrp.tile([128, 128], F32, tag="xr")
                    nc.gpsimd.indirect_dma_start(
                        out=xr[:],
                        out_offset=None,
                        in_=embed[:],
                        in_offset=IndirectOffsetOnAxis(
                            ap=idx_sb[:, g:g + 1], axis=0),
                    )
                    x_rows.append(xr)

                # ---------- P1: transpose (fwd + time-reversed) ----------
                with tc.tile_pool(name="xrel", bufs=4) as xrelp:
                    psts = [pq1, pq2]
                    for g in range(NBLK):
                        b_seq, tb4 = g // 4, g % 4
                        grev = b_seq * 4 + (3 - tb4)
                        xrel = xrelp.tile([128, 128], F32, tag="xrel")
                        nc.vector.tensor_tensor(
                            out=xrel[:], in0=x_rows[g][:], in1=x_rows[g][:],
                            op=Alu.max)
                        pst = psts[g % 2]
                        nc.tensor.transpose(
                            out=pst[:, 0:128], in_=xrel[:], identity=ident[:])
                        nc.vector.tensor_copy(
                            out=x_T[:, g * 128:(g + 1) * 128],
                            in_=pst[:, 0:128])
                        nc.tensor.transpose(
                            out=pst[:, 128:256], in_=xrel[:],
                            identity=rev_sb[:])
                        nc.vector.tensor_copy(
                            out=x_Trev[:, grev * 128:(grev + 1) * 128],
                            in_=pst[:, 128:256])

            # ---------- P2: bulk xproj (fp32r, N=512) ----------
            ps2s = [pw1[:], pw2[:], pw3[:]]
            n2 = 0
            for d in range(2):
                src_T = x_T if d == 0 else x_Trev
                for g in range(G4):
                    lhsT = wihr_sb[:, (d * G4 + g) * 128:(d * G4 + g + 1) * 128]
                    for b in range(Bc):
                        ps2 = ps2s[n2 % 3]
                        n2 += 1
                        nc.tensor.matmul(
                            out=ps2, lhsT=lhsT,
                            rhs=src_T[:, b * T:(b + 1) * T],
                            start=True, stop=True)
                        xp_sb = sm.tile([128, 512], XDT, tag="xp_out")
                        if d == 1 and g in (0, 1):
                            # poison bwd i/f gate lanes at padded steps
                            nc.vector.scalar_tensor_tensor(
                                out=xp_sb[:], in0=ps2,
                                scalar=bias_sb[:, d * G4 + g:d * G4 + g + 1],
                                in1=mpois_sb[:, b * T:(b + 1) * T],
                                op0=Alu.add, op1=Alu.add)
                        else:
                            nc.vector.tensor_scalar(
                                out=xp_sb[:], in0=ps2,
                                scalar1=bias_sb[:, d * G4 + g:d * G4 + g + 1],
                                scalar2=None, op0=Alu.add)
                        # store at PSUM block position (i,f,o,g order)
                        nc.sync.dma_start(
                            out=xproj_dram[d, GOFF[g] // 16, b],
                            in_=xp_sb[:])

            mpp_cm.__exit__(None, None, None)

            # ---------- P3: LSTM (all-tanh, C=2c / H=2h) ----------
            # h_all: slot r holds [H_f(t=r) | H_b(s=r)] fp16
            h_all = big.tile([128, T * 32], F16, tag="bigA")
            h0 = cst.tile([128, 32], F16)
            nc.vector.memset(h0[:], 0.0)
            c_st = cst.tile([128, 2 * Bc], F32)
            nc.vector.memset(c_st[:], 0.0)

            with tc.tile_pool(name="xpp", bufs=2) as xpp:
                for r in range(T):
                    c = r // CH
                    if r % CH == 0:
                        # layout [128, (y=(d, gslot, b), s)]: matches psum
                        # column order (d,g,b) when sliced at fixed s
                        xt = xpp.tile([128, CH * 128], XDT, tag="xpc")
                        for d in range(2):
                            dst = xt[:].rearrange(
                                "p (y s) -> p y s",
                                s=CH)[:, d * 64:(d + 1) * 64]
                            src = xproj_dram[:].rearrange(
                                "e g b p t -> p (e g b) t")[
                                :, d * 64:(d + 1) * 64,
                                c * CH:(c + 1) * CH]
                            nc.sync.dma_start(out=dst, in_=src)

                    ps3 = [pq1, pq2, pw3][r % 3]
                    # xproj into PSUM first (no h dependency), gates accum
                    xsl = xt[:].rearrange(
                        "p (y s) -> p y s", s=CH)[:, :, r % CH]
                    nc.tensor.matmul(
                        out=ps3[:, 0:128], lhsT=ident16[:], rhs=xsl,
                        start=True, stop=False)
                    hprev = h0[:] if r == 0 else h_all[:, (r - 1) * 32:r * 32]
                    for d in range(2):
                        for g in range(G4):
                            lhsT = whh_sb[
                                :, (d * G4 + g) * 128:(d * G4 + g + 1) * 128]
                            nc.tensor.matmul(
                                out=ps3[:, d * 64 + GOFF[g]:
                                        d * 64 + GOFF[g] + Bc],
                                lhsT=lhsT,
                                rhs=hprev[:, d * Bc:(d + 1) * Bc],
                                start=False, stop=(d == 1 and g == 3),
                                skip_group_check=True)

                    s_sb = sm.tile([128, 128], F32, tag="s_sb")
                    nc.scalar.activation(
                        out=s_sb[:], in_=ps3[:, 0:128], func=Act.Tanh)
                    s3 = s_sb[:].rearrange("p (d x) -> p d x", d=2)
                    t1 = sm.tile([128, 2 * Bc], F32, tag="t1")
                    nc.vector.scalar_tensor_tensor(
                        out=t1[:].rearrange("p (d x) -> p d x", d=2),
                        in0=s3[:, :, 0:16], scalar=1.0,
                        in1=s3[:, :, 48:64], op0=Alu.add, op1=Alu.mult)
                    t2 = sm.tile([128, 2 * Bc], F32, tag="t2")
                    nc.vector.scalar_tensor_tensor(
                        out=t2[:].rearrange("p (d x) -> p d x", d=2),
                        in0=s3[:, :, 16:32], scalar=1.0,
                        in1=c_st[:].rearrange("p (d x) -> p d x", d=2),
                        op0=Alu.add, op1=Alu.mult)
                    nc.vector.scalar_tensor_tensor(
                        out=c_st[:], in0=t2[:], scalar=0.5, in1=t1[:],
                        op0=Alu.mult, op1=Alu.add)
                    tcx = sm.tile([128, 2 * Bc], F32, tag="tc")
                    nc.scalar.activation(
                        out=tcx[:], in_=c_st[:], func=Act.Tanh, scale=0.5)
                    nc.vector.scalar_tensor_tensor(
                        out=h_all[:, r * 32:(r + 1) * 32].rearrange(
                            "p (d x) -> p d x", d=2),
                        in0=s3[:, :, 32:48], scalar=1.0,
                        in1=tcx[:].rearrange("p (d x) -> p d x", d=2),
                        op0=Alu.add, op1=Alu.mult)

            # ---------- P4: emissions em^T = [9, TOK] (fp16, N=512) ----------
            for blk in range(Bc):
                ps4 = [pw1, pw2][blk % 2][0:K, :]
                sl = slice(blk * T, (blk + 1) * T)
                hf_ap = h_all[:, blk::32]
                hb_ap = h_all[:, (T - 1) * 32 + 16 + blk:blk:-32]
                nc.tensor.matmul(
                    out=ps4, lhsT=woutTr_sb[:, 0:K], rhs=hf_ap,
                    start=True, stop=False)
                nc.tensor.matmul(
                    out=ps4, lhsT=woutTr_sb[:, K:2 * K], rhs=hb_ap,
                    start=False, stop=True)
                em_sb = sm.tile([K, T], F32, tag="em_sb")
                nc.vector.tensor_scalar(
                    out=em_sb[:], in0=ps4, scalar1=bout9_sb[:, 0:1],
                    scalar2=None, op0=Alu.add)
                nc.sync.dma_start(out=emT_dram[:, sl], in_=em_sb[:])

            # ---------- P5: blocked max-plus scan Viterbi ----------
            # partition p = b*8 + c: sequence b, time-chunk c (C=64 steps).
            # A_t[i,j] = mask_t ? trans[i,j] + em_t[j] : maxplus identity
            # (0 diag / -1e9 off); A_0 := I via maskA[.,0]=0 host fudge.
            # alpha_t = alpha_{t-1} (x) A_t  (row-vec max-plus)
            # beta_t  = A_{t+1} (x) beta_{t+1}  (col-vec max-plus)
            # tags_t  = argmax_i(alpha_t[i] + beta_t[i])
            C64, NC8 = 64, 8
            dpbig_cm = tc.tile_pool(name="dpbig", bufs=1)
            dbig = dpbig_cm.__enter__()

            em128 = dbig.tile([128, K * C64], F32, tag="em128")  # (j, k)
            nc.sync.dma_start(
                out=em128[:].rearrange("p (j k) -> p j k", j=K),
                in_=emT_dram[:].rearrange(
                    "k (b c t) -> (b c) k t", b=Bc, c=NC8))
            em0_sb = dbig.tile([Bc, K], F32, tag="em0")
            nc.sync.dma_start(
                out=em0_sb[:].unsqueeze(2),
                in_=emT_dram[:].rearrange("k (b t) -> b k t", b=Bc)[:, :, 0:1])

            trans128_sb = dbig.tile([128, 81], F32, tag="t128")
            nc.sync.dma_start(out=trans128_sb[:], in_=trans81f[:])
            identM128_sb = dbig.tile([128, 81], F32, tag="i128")
            nc.sync.dma_start(out=identM128_sb[:], in_=identM81f[:])
            maskA_sb = dbig.tile([128, C64], F32, tag="mA")
            nc.sync.dma_start(out=maskA_sb[:], in_=maskA[:])
            invmA_sb = dbig.tile([128, C64], F32, tag="imA")
            nc.sync.dma_start(out=invmA_sb[:], in_=invmA[:])
            tagm_sb = dbig.tile([128, C64], F32, tag="tagm")
            nc.sync.dma_start(out=tagm_sb[:], in_=tagm128[:])
            irev128_sb = dbig.tile([128, K], F32, tag="irev128")
            nc.sync.dma_start(out=irev128_sb[:], in_=irev128[:])

            # A slab build: A = m*(trans+em) + invm*I  (exact-zero masking)
            Aslab = dbig.tile([128, C64 * 81], F32, tag="Aslab")
            Atmp = dbig.tile([128, C64 * 81], F32, tag="Atmp")

            def A4(t_):
                return t_[:].rearrange("p (s i j) -> p s i j", s=C64, i=K)

            emv = em128[:].rearrange("p (j k) -> p k j", j=K)
            emv = emv.unsqueeze(2).to_broadcast([128, C64, K, K])
            t128v = trans128_sb[:].rearrange("p (i j) -> p i j", i=K)
            t128v = t128v.unsqueeze(1).to_broadcast([128, C64, K, K])
            i128v = identM128_sb[:].rearrange("p (i j) -> p i j", i=K)
            i128v = i128v.unsqueeze(1).to_broadcast([128, C64, K, K])
            mAv = maskA_sb[:].unsqueeze(2).unsqueeze(3).to_broadcast(
                [128, C64, K, K])
            imAv = invmA_sb[:].unsqueeze(2).unsqueeze(3).to_broadcast(
                [128, C64, K, K])
            nc.vector.tensor_tensor(out=A4(Atmp), in0=emv, in1=t128v,
                                    op=Alu.add)
            nc.vector.tensor_tensor(out=A4(Atmp), in0=A4(Atmp), in1=mAv,
                                    op=Alu.mult)
            nc.vector.tensor_tensor(out=A4(Aslab), in0=i128v, in1=imAv,
                                    op=Alu.mult)
            nc.vector.tensor_tensor(out=A4(Aslab), in0=A4(Aslab),
                                    in1=A4(Atmp), op=Alu.add)

            # transposed A copy: ATslab[s][(j,m)] = A_s[m,j] — makes every
            # level-1 operand innermost-contiguous (strided adds are ~1.7x)
            ATslab = big.tile([128, C64 * 81], F32, tag="bigB")
            nc.vector.tensor_copy(
                out=ATslab[:].rearrange("p (s j m) -> p s j m", s=C64, j=K),
                in_=A4(Aslab).transpose([0, 1, 3, 2]))

            # level 1: within-chunk prefix (Pslab) and suffix (SufT) products
            # Pslab[k] = A_0..A_k (i,j); SufT[k] = (A_k..A_63)^T (j,m)
            Pslab = dbig.tile([128, C64 * 81], F32, tag="Atmp")
            SufT = dbig.tile([128, (C64 + 1) * 81], F32, tag="Suf")
            candf = dbig.tile([128, 729], F32, tag="candf")
            candb = dbig.tile([128, 729], F32, tag="candb")
            nc.vector.tensor_copy(out=SufT[:, C64 * 81:], in_=identM128_sb[:])
            nc.vector.tensor_copy(out=Pslab[:, 0:81], in_=Aslab[:, 0:81])

            def m81(ap, kk):
                return ap[:, kk * 81:(kk + 1) * 81]

            def as_ij(ap3):
                return ap3.rearrange("p (i j) -> p i j", i=K)

            def jm(ap81):
                return ap81.rearrange("p (j m) -> p j m", j=K)

            def sufT_out(kk):
                # write (i,j)-indexed result into (j,i) storage
                return m81(SufT, kk).rearrange(
                    "p (j i) -> p j i", j=K).transpose([0, 2, 1])

            def bwd_step(kb):
                av2 = as_ij(m81(Aslab, kb)).unsqueeze(2).to_broadcast(
                    [128, K, K, K])                          # (i, j, m)
                sv = jm(m81(SufT, kb + 1)).unsqueeze(1).to_broadcast(
                    [128, K, K, K])                          # (i, j, m)
                nc.vector.tensor_tensor(
                    out=candb[:].rearrange("p (i j m) -> p i j m", i=K, j=K),
                    in0=av2, in1=sv, op=Alu.add)
                nc.vector.tensor_reduce(
                    out=sufT_out(kb),
                    in_=candb[:].rearrange("p (i j m) -> p i j m", i=K, j=K),
                    axis=AxX, op=Alu.max)

            for k in range(1, C64):
                # interleave fwd step k and bwd step (C64-k) on DVE
                pv = as_ij(m81(Pslab, k - 1)).unsqueeze(2).to_broadcast(
                    [128, K, K, K])                          # (i, j, m)
                av = jm(m81(ATslab, k)).unsqueeze(1).to_broadcast(
                    [128, K, K, K])                          # (i, j, m)
                nc.vector.tensor_tensor(
                    out=candf[:].rearrange("p (i j m) -> p i j m", i=K, j=K),
                    in0=pv, in1=av, op=Alu.add)
                nc.vector.tensor_reduce(
                    out=as_ij(m81(Pslab, k)),
                    in_=candf[:].rearrange("p (i j m) -> p i j m", i=K, j=K),
                    axis=AxX, op=Alu.max)
                bwd_step(C64 - k)
            bwd_step(0)

            # level 2: cross-chunk alpha-start / beta-end chains on p0-15
            Gg = dbig.tile([Bc, NC8 * 81], F32, tag="Gg")
            nc.sync.dma_start(out=Gg[:], in_=m81(Pslab, C64 - 1))
            Sg = dbig.tile([Bc, NC8 * 81], F32, tag="Sg")
            nc.sync.dma_start(out=Sg[:], in_=m81(SufT, 0))
            ast = dbig.tile([Bc, NC8 * K], F32, tag="ast")
            bend = dbig.tile([Bc, NC8 * K], F32, tag="bend")
            candL = dbig.tile([Bc, 81], F32, tag="candL")
            nc.vector.tensor_tensor(
                out=ast[:, 0:K], in0=start9_sb[:], in1=em0_sb[:], op=Alu.add)
            for c in range(1, NC8):
                # ast_c[j] = max_i ast_{c-1}[i] + G_{c-1}[i,j]
                in0 = ast[:, (c - 1) * K:c * K].unsqueeze(1).to_broadcast(
                    [Bc, K, K])                               # (j, i)
                in1 = Gg[:, (c - 1) * 81:c * 81].rearrange(
                    "b (i j) -> b i j", i=K).transpose([0, 2, 1])
                nc.vector.tensor_tensor(
                    out=candL[:].rearrange("b (j i) -> b j i", j=K),
                    in0=in0, in1=in1, op=Alu.add)
                nc.vector.tensor_reduce(
                    out=ast[:, c * K:(c + 1) * K],
                    in_=candL[:].rearrange("b (j i) -> b j i", j=K),
                    axis=AxX, op=Alu.max)
            nc.vector.tensor_copy(
                out=bend[:, (NC8 - 1) * K:], in_=end9_sb[:])
            for c in range(NC8 - 2, -1, -1):
                # bend_c[i] = max_j S0_{c+1}[i,j] + bend_{c+1}[j]
                in0 = bend[:, (c + 1) * K:(c + 2) * K].unsqueeze(1) \
                    .to_broadcast([Bc, K, K])                 # (i, j)
                in1 = Sg[:, (c + 1) * 81:(c + 2) * 81].rearrange(
                    "b (j i) -> b j i", j=K).transpose([0, 2, 1])
                nc.vector.tensor_tensor(
                    out=candL[:].rearrange("b (i j) -> b i j", i=K),
                    in0=in0, in1=in1, op=Alu.add)
                nc.vector.tensor_reduce(
                    out=bend[:, c * K:(c + 1) * K],
                    in_=candL[:].rearrange("b (i j) -> b i j", i=K),
                    axis=AxX, op=Alu.max)
            alpha_start = dbig.tile([128, K], F32, tag="astart")
            nc.sync.dma_start(out=alpha_start[:], in_=ast[:])
            beta_end = dbig.tile([128, K], F32, tag="bstart")
            nc.sync.dma_start(out=beta_end[:], in_=bend[:])

            # level 3: all per-step alpha/beta via one bulk op pair each
            cand3 = dbig.tile([128, C64 * 81], F32, tag="Aslab")
            alpha_all = dbig.tile([128, C64 * K], F32, tag="em128")
            in0 = alpha_start[:].unsqueeze(1).unsqueeze(1).to_broadcast(
                [128, C64, K, K])                             # (k, j, i)
            in1 = Pslab[:].rearrange(
                "p (k i j) -> p k i j", k=C64, i=K).transpose([0, 1, 3, 2])
            nc.vector.tensor_tensor(
                out=cand3[:].rearrange("p (k j i) -> p k j i", k=C64, j=K),
                in0=in0, in1=in1, op=Alu.add)
            nc.vector.tensor_reduce(
                out=alpha_all[:].rearrange("p (k j) -> p k j", k=C64),
                in_=cand3[:].rearrange("p (k j i) -> p k j i", k=C64, j=K),
                axis=AxX, op=Alu.max)
            cand4 = dbig.tile([128, C64 * 81], F32, tag="Atmp")
            beta_all = dbig.tile([128, C64 * K], F32, tag="ball")
            in0 = beta_end[:].unsqueeze(1).unsqueeze(1).to_broadcast(
                [128, C64, K, K])                             # (k, i, j)
            in1 = SufT[:, 81:].rearrange(
                "p (k j i) -> p k j i", k=C64, j=K).transpose([0, 1, 3, 2])
            nc.vector.tensor_tensor(
                out=cand4[:].rearrange("p (k i j) -> p k i j", k=C64, i=K),
                in0=in0, in1=in1, op=Alu.add)
            nc.vector.tensor_reduce(
                out=beta_all[:].rearrange("p (k i) -> p k i", k=C64),
                in_=cand4[:].rearrange("p (k i j) -> p k i j", k=C64, i=K),
                axis=AxX, op=Alu.max)

            # ---------- P6: tags = argmax_i(alpha + beta), mask, emit ------
            nc.vector.tensor_tensor(
                out=alpha_all[:], in0=alpha_all[:], in1=beta_all[:],
                op=Alu.add)
            mx128 = dbig.tile([128, C64], F32, tag="mx128")
            nc.vector.tensor_reduce(
                out=mx128[:],
                in_=alpha_all[:].rearrange("p (k j) -> p k j", k=C64),
                axis=AxX, op=Alu.max)
            nc.vector.tensor_tensor(
                out=beta_all[:].rearrange("p (k j) -> p k j", k=C64),
                in0=alpha_all[:].rearrange("p (k j) -> p k j", k=C64),
                in1=mx128[:].unsqueeze(2).to_broadcast([128, C64, K]),
                op=Alu.is_equal)
            nc.vector.tensor_tensor(
                out=beta_all[:].rearrange("p (k j) -> p k j", k=C64),
                in0=beta_all[:].rearrange("p (k j) -> p k j", k=C64),
                in1=irev128_sb[:].unsqueeze(1).to_broadcast([128, C64, K]),
                op=Alu.mult)
            tags128 = dbig.tile([128, C64], F32, tag="tags128")
            nc.vector.tensor_reduce(
                out=tags128[:],
                in_=beta_all[:].rearrange("p (k j) -> p k j", k=C64),
                axis=AxX, op=Alu.max)
            nc.vector.tensor_scalar(
                out=tags128[:], in0=tags128[:], scalar1=-1.0, scalar2=8.0,
                op0=Alu.mult, op1=Alu.add)
            nc.vector.tensor_tensor(
                out=tags128[:], in0=tags128[:], in1=tagm_sb[:], op=Alu.mult)
            tagsi = dbig.tile([128, C64], I32, tag="tagsi")
            nc.vector.tensor_copy(out=tagsi[:], in_=tags128[:])
            nc.sync.dma_start(out=tags_out[:], in_=tagsi[:])
            dpbig_cm.__exit__(None, None, None)
            psp_cm.__exit__(None, None, None)

    nc.finalize()
    return nc


_NC_CACHE = None


def _get_program():
    global _NC_CACHE
    if _NC_CACHE is None:
        _NC_CACHE = build_program()
    return _NC_CACHE


def make_in_maps(sentences, lengths, embed, Wih_f, Whh_f, bih_f, bhh_f,
                 Wih_b, Whh_b, bih_b, bhh_b, W_out, b_out, start_t, end_t,
                 trans):
    sentences = np.ascontiguousarray(sentences, dtype=np.int32)
    embed = np.ascontiguousarray(embed, dtype=np.float32)
    lengths = np.asarray(lengths)

    # all-tanh pre-scaling: s = tanh(psum) must give tanh(x/2) for i,f,o
    # (gate blocks 0,1,3) and tanh(g) for block 2; h is stored as H=2h so
    # Whh additionally absorbs a 0.5.
    whh_pack = np.zeros((128, 1024), np.float32)
    wih_pack = np.zeros((128, 1024), np.float32)
    bias_pack = np.zeros((128, 8), np.float32)
    for d, (Wih, Whh, bi, bh) in enumerate(
            ((Wih_f, Whh_f, bih_f, bhh_f), (Wih_b, Whh_b, bih_b, bhh_b))):
        for g in range(G4):
            s_ih = 0.5 if g in (0, 1, 3) else 1.0
            s_hh = 0.25 if g in (0, 1, 3) else 0.5
            whh_pack[:, (d * G4 + g) * 128:(d * G4 + g + 1) * 128] = \
                np.asarray(Whh)[g * 128:(g + 1) * 128, :].T * s_hh
            wih_pack[:, (d * G4 + g) * 128:(d * G4 + g + 1) * 128] = \
                np.asarray(Wih)[g * 128:(g + 1) * 128, :].T * s_ih
            bias_pack[:, d * G4 + g] = \
                (np.asarray(bi) + np.asarray(bh))[g * 128:(g + 1) * 128] * s_ih

    W_out = np.asarray(W_out, np.float32) * 0.5  # H = 2h compensation
    woutT = np.zeros((128, 18), np.float32)
    woutT[:, 0:K] = W_out[:, :128].T
    woutT[:, K:2 * K] = W_out[:, 128:].T
    bout9 = np.asarray(b_out, np.float32)[:, None].copy()

    rev_id = np.zeros((128, 128), np.float32)
    rev_id[np.arange(128), 127 - np.arange(128)] = 1.0

    trans_np = np.asarray(trans, np.float32)
    identM = np.full((K, K), NEG, np.float32)
    np.fill_diagonal(identM, 0.0)
    trans81f = np.broadcast_to(trans_np.reshape(-1)[None], (128, 81)).copy()
    identM81f = np.broadcast_to(identM.reshape(-1)[None], (128, 81)).copy()

    start9 = np.broadcast_to(
        np.asarray(start_t, np.float32)[None, :], (Bc, K)).copy()
    end9 = np.broadcast_to(
        np.asarray(end_t, np.float32)[None, :], (Bc, K)).copy()
    ii = np.arange(K, dtype=np.float32)
    irev128 = np.broadcast_to((8.0 - ii)[None, :], (128, K)).copy()
    tt = np.arange(T)

    in_maps = []
    for c in range(NC):
        sl = slice(c * Bc, (c + 1) * Bc)
        sents_c = sentences[sl]
        lens_c = np.asarray(lengths[sl], np.float32)
        idx_np = np.zeros((128, NBLK), np.int32)
        p = np.arange(128)
        for g in range(NBLK):
            bt = g * 128 + p
            idx_np[:, g] = sents_c[bt // T, bt % T]
        mask_np = (tt[None, :] < lens_c[:, None]).astype(np.float32)
        # bwd i/f poison: s-index s corresponds to t = T-1-s; padded when
        # t >= len i.e. s < T - len
        ss = np.arange(T)
        mpois = np.zeros((Bc, T), np.float32)
        for b in range(Bc):
            mpois[b, ss < T - int(lens_c[b])] = POIS
        mpois128 = np.broadcast_to(
            mpois.reshape(1, -1), (128, TOK)).astype(np.float16).copy()
        # [128, 64] chunked masks, partition p = b*8 + c
        tagm = mask_np.reshape(Bc * 8, 64).copy()
        maskA_np = tagm.copy()
        maskA_np[0::8, 0] = 0.0  # A_0 := maxplus identity
        in_maps.append({
            "embed": embed,
            "idx": idx_np,
            "whh_pack": whh_pack.astype(np.float16),
            "wih_pack": wih_pack, "bias_pack": bias_pack,
            "mpois": mpois128,
            "woutT": woutT, "bout9": bout9, "rev_id": rev_id,
            "trans81f": trans81f, "identM81f": identM81f,
            "maskA": maskA_np, "invmA": 1.0 - maskA_np, "tagm128": tagm,
            "irev128": irev128,
            "start9": start9, "end9": end9,
        })
    return in_maps


def run(inputs, trace=False, **kw):
    nc = _get_program()
    in_maps = make_in_maps(**inputs)
    res = run_bass_kernel_spmd(nc, in_maps, list(range(NC)), trace=trace, **kw)
    tags = np.concatenate([r["tags"] for r in res.results], axis=0)
    return tags.astype(np.int32), res


def kernel(**inputs):
    tags, _ = run(inputs)
    return tags


# revision 19
# speedup vs baseline: 1.3469x; 1.0556x over previous
"""BiLSTM-CRF Viterbi decode on 8 Trainium2 NeuronCores.

Data-parallel over batch: each core handles 16 of 128 sequences.

Per-core phases:
  P0 embedding gather (indirect DMA, 128 rows per DMA)
  P1 PE-transpose x_rows [tok,E] -> x_T [E,tok] and x_Trev (time-reversed
     per sequence, via anti-diagonal identity)
  P2 bulk input projection xproj = Wih_g @ x_T (+bias) staged to DRAM fp16,
     fp32r matmuls (N=512); bwd direction projected from x_Trev so its
     DRAM layout is s-ordered (s = T-1-t). Bwd i/f gate lanes at padded
     steps are poisoned to -1e4 so tanh saturates to -1 and the cell
     state provably stays zero through the padded prefix (replaces all
     per-round masking).
  P3 512 fused fwd+bwd LSTM rounds, all-tanh formulation:
     host pre-scales weights so ONE tanh over the [128,128] gate PSUM
     yields s = tanh(x/2) for i,f,o (i.e. 2*sigmoid-1) and tanh(g) for g.
     Cell state kept as C = 2c, hidden as H = 2h:
        t1 = (s_i+1)*tg          (= 2 sig_i tg)
        t2 = (s_f+1)*C           (= 2 sig_f C)
        C' = t2*0.5 + t1         (= 2c')
        tcx = tanh(0.5*C')       (= tanh(c'))
        H' = (s_o+1)*tcx         (= 2h')
     xproj joins the gates via an fp16 identity matmul accumulating into
     the same PSUM bank (no DVE add). Both directions' H live in one
     [128,32] slot per round (bwd s-ordered) so one STT writes both and
     all 8 recurrence matmuls wait on a single semaphore.
  P4 emissions em^T = W_out @ h as [9,512] fp16 matmuls (strided /
     reverse-strided rhs straight out of h_all) staged to DRAM
  P5 blocked max-plus scan Viterbi: time-chunks on partitions
     (p = b*8 + c, 64 steps/chunk), within-chunk prefix/suffix 9x9
     max-plus matrix products on DVE, tiny cross-chunk chains, then one
     bulk op pair recovers every alpha_t / beta_t
  P6 tags_t = argmax_i(alpha_t + beta_t), bulk [128, 64*9] ops

Precision: xproj fp32r matmul staged fp16, recurrence fp16, DP fp32.
"""

import ml_dtypes
import numpy as np

import concourse.bacc as bacc
import concourse.bass as bass
import concourse.mybir as mybir
import concourse.tile as tile
from concourse.bass import IndirectOffsetOnAxis
from concourse.bass_utils import run_bass_kernel_spmd
from concourse.masks import make_identity

F32 = mybir.dt.float32
F32R = mybir.dt.float32r
BF16 = mybir.dt.bfloat16
F16 = mybir.dt.float16
I32 = mybir.dt.int32
Alu = mybir.AluOpType
Act = mybir.ActivationFunctionType
AxX = mybir.AxisListType.X

B, T, V, E, H, K = 128, 512, 100000, 128, 128, 9
NC = 8
Bc = B // NC          # 16 sequences per core
TOK = Bc * T          # 8192 tokens per core, flat index bt = b*T + t (b-major)
NBLK = TOK // 128     # 64 gather/transpose blocks
G4 = 4
# gate order in weights: i, f, g, o (torch). psum cols per dir: i(0) f(16) o(32) g(48)
GOFF = {0: 0, 1: 16, 3: 32, 2: 48}
CH = 32               # LSTM rounds per xproj chunk
NCH = T // CH
NEG = -1.0e9
POIS = -1.0e4         # bwd i/f gate poison at padded steps
XDT = F16             # xproj staging dtype


def f32(ap):
    return ap.bitcast(F32)


def build_program():
    nc = bacc.Bacc(None, target_bir_lowering=False)

    # ---------------- dram parameters ----------------
    embed = nc.declare_dram_parameter("embed", [V, E], F32, isOutput=False)
    idx = nc.declare_dram_parameter("idx", [128, NBLK], I32, isOutput=False)
    whh_pack = nc.declare_dram_parameter("whh_pack", [128, 1024], F16, isOutput=False)
    wih_pack = nc.declare_dram_parameter("wih_pack", [128, 1024], F32, isOutput=False)
    bias_pack = nc.declare_dram_parameter("bias_pack", [128, 8], F32, isOutput=False)
    mpois = nc.declare_dram_parameter("mpois", [128, TOK], F16, isOutput=False)
    woutT = nc.declare_dram_parameter("woutT", [128, 18], F32, isOutput=False)
    bout9 = nc.declare_dram_parameter("bout9", [K, 1], F32, isOutput=False)
    rev_id = nc.declare_dram_parameter("rev_id", [128, 128], F32, isOutput=False)
    trans81f = nc.declare_dram_parameter("trans81f", [128, 81], F32, isOutput=False)
    identM81f = nc.declare_dram_parameter("identM81f", [128, 81], F32, isOutput=False)
    maskA = nc.declare_dram_parameter("maskA", [128, 64], F32, isOutput=False)
    invmA = nc.declare_dram_parameter("invmA", [128, 64], F32, isOutput=False)
    tagm128 = nc.declare_dram_parameter("tagm128", [128, 64], F32, isOutput=False)
    irev128 = nc.declare_dram_parameter("irev128", [128, K], F32, isOutput=False)
    start9 = nc.declare_dram_parameter("start9", [Bc, K], F32, isOutput=False)
    end9 = nc.declare_dram_parameter("end9", [Bc, K], F32, isOutput=False)
    tags_out = nc.declare_dram_parameter("tags", [Bc, T], I32, isOutput=True)

    # ---------------- dram internals ----------------
    xproj_dram = nc.dram_tensor("xproj_dram", [2, G4, Bc, 128, T], XDT)
    emT_dram = nc.dram_tensor("emT_dram", [K, TOK], F32)

    with tile.TileContext(nc) as tc:
        with (
            tc.tile_pool(name="big", bufs=1) as big,
            tc.tile_pool(name="consts", bufs=1) as cst,
            tc.tile_pool(name="small", bufs=4) as sm,
        ):
            # ---------- constants ----------
            idx_sb = cst.tile([128, NBLK], I32)
            nc.sync.dma_start(out=idx_sb[:], in_=idx[:])
            whh_sb = cst.tile([128, 1024], F16)
            nc.sync.dma_start(out=whh_sb[:], in_=whh_pack[:])
            wih_sb = cst.tile([128, 1024], F32)
            nc.sync.dma_start(out=wih_sb[:], in_=wih_pack[:])
            bias_sb = cst.tile([128, 8], F32)
            nc.sync.dma_start(out=bias_sb[:], in_=bias_pack[:])
            mpp_cm = tc.tile_pool(name="mpp", bufs=1)
            mpp = mpp_cm.__enter__()
            mpois_sb = mpp.tile([128, TOK], F16)
            nc.sync.dma_start(out=mpois_sb[:], in_=mpois[:])
            woutT_sb = cst.tile([128, 18], F32)
            nc.sync.dma_start(out=woutT_sb[:], in_=woutT[:])
            # device-rounded fp32r copies (the fp32r matmul path requires
            # its operands to be produced pre-rounded to fp32r)
            wihr_sb = cst.tile([128, 1024], F32R)
            nc.vector.tensor_copy(out=wihr_sb[:], in_=wih_sb[:])
            woutTr_sb = cst.tile([128, 18], F16)
            nc.vector.tensor_copy(out=woutTr_sb[:], in_=woutT_sb[:])
            bout9_sb = cst.tile([K, 1], F32)
            nc.sync.dma_start(out=bout9_sb[:], in_=bout9[:])
            rev_dma = cst.tile([128, 128], F32)
            nc.sync.dma_start(out=rev_dma[:], in_=rev_id[:])
            rev_sb = cst.tile([128, 128], F32)
            nc.vector.tensor_copy(out=rev_sb[:], in_=rev_dma[:])
            ident = cst.tile([128, 128], F32)
            make_identity(nc, ident[:])
            ident16 = cst.tile([128, 128], F16)
            nc.vector.tensor_copy(out=ident16[:], in_=ident[:])

            # DP constants (p0-15)
            start9_sb = cst.tile([Bc, K], F32)
            nc.sync.dma_start(out=start9_sb[:], in_=start9[:])
            end9_sb = cst.tile([Bc, K], F32)
            nc.sync.dma_start(out=end9_sb[:], in_=end9[:])

            # PE "absorber" ops: self-loading (fp32/fp32r) matmuls may carry
            # at most ONE sync wait in walrus codegen. These tiny ops advance
            # PE's vector clock over one-time deps (identity from Pool,
            # const-weight DMA lanes) so real matmuls each need <=1 wait.
            psp_cm = tc.tile_pool(name="psglob", bufs=1, space="PSUM")
            psp = psp_cm.__enter__()
            pq1 = psp.tile([128, 512], F32, tag="pq1", name="pq1")
            pq2 = psp.tile([128, 512], F32, tag="pq2", name="pq2")
            pw1 = psp.tile([128, 512], F32, tag="pw1", name="pw1")
            pw2 = psp.tile([128, 512], F32, tag="pw2", name="pw2")
            pw3 = psp.tile([128, 512], F32, tag="pw3", name="pw3")
            nc.tensor.transpose(out=pq1[:, 0:128], in_=ident[:], identity=ident[:])
            nc.tensor.transpose(out=pq2[:, 0:128], in_=rev_sb[:], identity=ident[:])
            nc.tensor.matmul(out=pq2[0:1, 0:1], lhsT=whh_sb[:, 0:1],
                             rhs=whh_sb[:, 0:1], start=True, stop=True)
            nc.tensor.matmul(out=pq1[0:1, 0:1], lhsT=ident16[:, 0:1],
                             rhs=whh_sb[:, 0:1], start=True, stop=True)

            # ---------- P0: gather ----------
            x_T = big.tile([128, TOK], F32R, tag="bigA")
            with tc.tile_pool(name="xr", bufs=24) as xrp:
                x_rows = []
                for g in range(NBLK):
                    xr = xrp.tile([128, 128], F32, tag="xr")
                    nc.gpsimd.indirect_dma_start(
                        out=xr[:],
                        out_offset=None,
                        in_=embed[:],
                        in_offset=IndirectOffsetOnAxis(
                            ap=idx_sb[:, g:g + 1], axis=0),
                    )
                    x_rows.append(xr)

                # ---------- P1: transpose (fwd only; bwd read reversed) ----
                with tc.tile_pool(name="xrel", bufs=4) as xrelp:
                    psts = [pq1, pq2]
                    for g in range(NBLK):
                        xrel = xrelp.tile([128, 128], F32, tag="xrel")
                        nc.vector.tensor_tensor(
                            out=xrel[:], in0=x_rows[g][:], in1=x_rows[g][:],
                            op=Alu.max)
                        pst = psts[g % 2]
                        nc.tensor.transpose(
                            out=pst[:, 0:128], in_=xrel[:], identity=ident[:])
                        if g % 2 == 0:
                            nc.vector.tensor_copy(
                                out=x_T[:, g * 128:(g + 1) * 128],
                                in_=pst[:, 0:128])
                        else:
                            nc.scalar.activation(
                                out=x_T[:, g * 128:(g + 1) * 128],
                                in_=pst[:, 0:128], func=Act.Copy)

            # ---------- P2: bulk xproj (fp32r, N=512) ----------
            # bwd (d=1) reads x_T with reversed t so its DRAM layout is
            # s-ordered without needing a second transposed copy
            xv = x_T[:].rearrange("p (b t) -> p b t", b=Bc)
            ps2s = [pw1[:], pw2[:], pw3[:]]
            n2 = 0
            for d in range(2):
                for g in range(G4):
                    lhsT = wihr_sb[:, (d * G4 + g) * 128:(d * G4 + g + 1) * 128]
                    for b in range(Bc):
                        ps2 = ps2s[n2 % 3]
                        n2 += 1
                        rhs = xv[:, b] if d == 0 else xv[:, b, ::-1]
                        nc.tensor.matmul(
                            out=ps2, lhsT=lhsT, rhs=rhs,
                            start=True, stop=True)
                        xp_sb = sm.tile([128, 512], XDT, tag="xp_out")
                        if d == 1 and g in (0, 1):
                            # poison bwd i/f gate lanes at padded steps
                            nc.vector.scalar_tensor_tensor(
                                out=xp_sb[:], in0=ps2,
                                scalar=bias_sb[:, d * G4 + g:d * G4 + g + 1],
                                in1=mpois_sb[:, b * T:(b + 1) * T],
                                op0=Alu.add, op1=Alu.add)
                        elif n2 % 2 == 0:
                            nc.vector.tensor_scalar(
                                out=xp_sb[:], in0=ps2,
                                scalar1=bias_sb[:, d * G4 + g:d * G4 + g + 1],
                                scalar2=None, op0=Alu.add)
                        else:
                            nc.scalar.activation(
                                out=xp_sb[:], in_=ps2, func=Act.Identity,
                                bias=bias_sb[:, d * G4 + g:d * G4 + g + 1])
                        # store at PSUM block position (i,f,o,g order)
                        nc.sync.dma_start(
                            out=xproj_dram[d, GOFF[g] // 16, b],
                            in_=xp_sb[:])

            mpp_cm.__exit__(None, None, None)

            # ---------- P3: LSTM (all-tanh, C=2c / H=2h) ----------
            # h_all: slot r holds [H_f(t=r) | H_b(s=r)] fp16
            h_all = big.tile([128, T * 32], F16, tag="bigA")
            h0 = cst.tile([128, 32], F16)
            nc.vector.memset(h0[:], 0.0)
            c_st = cst.tile([128, 2 * Bc], F32)
            nc.vector.memset(c_st[:], 0.0)

            with tc.tile_pool(name="xpp", bufs=2) as xpp:
                for r in range(T):
                    c = r // CH
                    if r % CH == 0:
                        # layout [128, (y=(d, gslot, b), s)]: matches psum
                        # column order (d,g,b) when sliced at fixed s
                        xt = xpp.tile([128, CH * 128], XDT, tag="xpc")
                        for d in range(2):
                            dst = xt[:].rearrange(
                                "p (y s) -> p y s",
                                s=CH)[:, d * 64:(d + 1) * 64]
                            src = xproj_dram[:].rearrange(
                                "e g b p t -> p (e g b) t")[
                                :, d * 64:(d + 1) * 64,
                                c * CH:(c + 1) * CH]
                            nc.sync.dma_start(out=dst, in_=src)

                    ps3 = [pq1, pq2, pw3][r % 3]
                    # xproj into PSUM first (no h dependency), gates accum
                    xsl = xt[:].rearrange(
                        "p (y s) -> p y s", s=CH)[:, :, r % CH]
                    nc.tensor.matmul(
                        out=ps3[:, 0:128], lhsT=ident16[:], rhs=xsl,
                        start=True, stop=False)
                    hprev = h0[:] if r == 0 else h_all[:, (r - 1) * 32:r * 32]
                    for d in range(2):
                        for g in range(G4):
                            lhsT = whh_sb[
                                :, (d * G4 + g) * 128:(d * G4 + g + 1) * 128]
                            nc.tensor.matmul(
                                out=ps3[:, d * 64 + GOFF[g]:
                                        d * 64 + GOFF[g] + Bc],
                                lhsT=lhsT,
                                rhs=hprev[:, d * Bc:(d + 1) * Bc],
                                start=False, stop=(d == 1 and g == 3),
                                skip_group_check=True)

                    s_sb = sm.tile([128, 128], F32, tag="s_sb")
                    nc.scalar.activation(
                        out=s_sb[:], in_=ps3[:, 0:128], func=Act.Tanh)
                    s3 = s_sb[:].rearrange("p (d x) -> p d x", d=2)
                    t1 = sm.tile([128, 2 * Bc], F32, tag="t1")
                    nc.vector.scalar_tensor_tensor(
                        out=t1[:].rearrange("p (d x) -> p d x", d=2),
                        in0=s3[:, :, 0:16], scalar=1.0,
                        in1=s3[:, :, 48:64], op0=Alu.add, op1=Alu.mult)
                    t2 = sm.tile([128, 2 * Bc], F32, tag="t2")
                    nc.vector.scalar_tensor_tensor(
                        out=t2[:].rearrange("p (d x) -> p d x", d=2),
                        in0=s3[:, :, 16:32], scalar=1.0,
                        in1=c_st[:].rearrange("p (d x) -> p d x", d=2),
                        op0=Alu.add, op1=Alu.mult)
                    nc.vector.scalar_tensor_tensor(
                        out=c_st[:], in0=t2[:], scalar=0.5, in1=t1[:],
                        op0=Alu.mult, op1=Alu.add)
                    tcx = sm.tile([128, 2 * Bc], F32, tag="tc")
                    nc.scalar.activation(
                        out=tcx[:], in_=c_st[:], func=Act.Tanh, scale=0.5)
                    nc.vector.scalar_tensor_tensor(
                        out=h_all[:, r * 32:(r + 1) * 32].rearrange(
                            "p (d x) -> p d x", d=2),
                        in0=s3[:, :, 32:48], scalar=1.0,
                        in1=tcx[:].rearrange("p (d x) -> p d x", d=2),
                        op0=Alu.add, op1=Alu.mult)

            # ---------- P4: emissions em^T = [9, TOK] (fp16, N=512) ----------
            for blk in range(Bc):
                ps4 = [pw1, pw2][blk % 2][0:K, :]
                sl = slice(blk * T, (blk + 1) * T)
                hf_ap = h_all[:, blk::32]
                hb_ap = h_all[:, (T - 1) * 32 + 16 + blk:blk:-32]
                nc.tensor.matmul(
                    out=ps4, lhsT=woutTr_sb[:, 0:K], rhs=hf_ap,
                    start=True, stop=False)
                nc.tensor.matmul(
                    out=ps4, lhsT=woutTr_sb[:, K:2 * K], rhs=hb_ap,
                    start=False, stop=True)
                em_sb = sm.tile([K, T], F32, tag="em_sb")
                nc.vector.tensor_scalar(
                    out=em_sb[:], in0=ps4, scalar1=bout9_sb[:, 0:1],
                    scalar2=None, op0=Alu.add)
                nc.sync.dma_start(out=emT_dram[:, sl], in_=em_sb[:])

            # ---------- P5: blocked max-plus scan Viterbi ----------
            # partition p = b*8 + c: sequence b, time-chunk c (C=64 steps).
            # A_t[i,j] = mask_t ? trans[i,j] + em_t[j] : maxplus identity
            # (0 diag / -1e9 off); A_0 := I via maskA[.,0]=0 host fudge.
            # alpha_t = alpha_{t-1} (x) A_t  (row-vec max-plus)
            # beta_t  = A_{t+1} (x) beta_{t+1}  (col-vec max-plus)
            # tags_t  = argmax_i(alpha_t[i] + beta_t[i])
            C64, NC8 = 64, 8
            dpbig_cm = tc.tile_pool(name="dpbig", bufs=1)
            dbig = dpbig_cm.__enter__()

            em128 = dbig.tile([128, K * C64], F32, tag="em128")  # (j, k)
            nc.sync.dma_start(
                out=em128[:].rearrange("p (j k) -> p j k", j=K),
                in_=emT_dram[:].rearrange(
                    "k (b c t) -> (b c) k t", b=Bc, c=NC8))
            em0_sb = dbig.tile([Bc, K], F32, tag="em0")
            nc.sync.dma_start(
                out=em0_sb[:].unsqueeze(2),
                in_=emT_dram[:].rearrange("k (b t) -> b k t", b=Bc)[:, :, 0:1])

            trans128_sb = dbig.tile([128, 81], F32, tag="t128")
            nc.sync.dma_start(out=trans128_sb[:], in_=trans81f[:])
            identM128_sb = dbig.tile([128, 81], F32, tag="i128")
            nc.sync.dma_start(out=identM128_sb[:], in_=identM81f[:])
            maskA_sb = dbig.tile([128, C64], F32, tag="mA")
            nc.sync.dma_start(out=maskA_sb[:], in_=maskA[:])
            invmA_sb = dbig.tile([128, C64], F32, tag="imA")
            nc.sync.dma_start(out=invmA_sb[:], in_=invmA[:])
            tagm_sb = dbig.tile([128, C64], F32, tag="tagm")
            nc.sync.dma_start(out=tagm_sb[:], in_=tagm128[:])
            irev128_sb = dbig.tile([128, K], F32, tag="irev128")
            nc.sync.dma_start(out=irev128_sb[:], in_=irev128[:])

            # A slab build: A = m*(trans+em) + invm*I  (exact-zero masking)
            Aslab = dbig.tile([128, C64 * 81], F32, tag="Aslab")
            Atmp = dbig.tile([128, C64 * 81], F32, tag="Atmp")

            def A4(t_):
                return t_[:].rearrange("p (s i j) -> p s i j", s=C64, i=K)

            emv = em128[:].rearrange("p (j k) -> p k j", j=K)
            emv = emv.unsqueeze(2).to_broadcast([128, C64, K, K])
            t128v = trans128_sb[:].rearrange("p (i j) -> p i j", i=K)
            t128v = t128v.unsqueeze(1).to_broadcast([128, C64, K, K])
            i128v = identM128_sb[:].rearrange("p (i j) -> p i j", i=K)
            i128v = i128v.unsqueeze(1).to_broadcast([128, C64, K, K])
            mAv = maskA_sb[:].bitcast(I32).unsqueeze(2).unsqueeze(3) \
                .to_broadcast([128, C64, K, K])
            imAv = invmA_sb[:].unsqueeze(2).unsqueeze(3).to_broadcast(
                [128, C64, K, K])
            nc.vector.tensor_tensor(out=A4(Atmp), in0=emv, in1=t128v,
                                    op=Alu.add)
            nc.vector.tensor_copy(out=A4(Aslab), in_=i128v)
            nc.vector.copy_predicated(out=A4(Aslab), mask=mAv, data=A4(Atmp))

            # transposed A copy: ATslab[s][(j,m)] = A_s[m,j] — makes every
            # level-1 operand innermost-contiguous (strided adds are ~1.7x)
            ATslab = big.tile([128, C64 * 81], F32, tag="bigB")
            nc.vector.tensor_copy(
                out=ATslab[:].rearrange("p (s j m) -> p s j m", s=C64, j=K),
                in_=A4(Aslab).transpose([0, 1, 3, 2]))

            # level 1: within-chunk prefix products only; the per-step
            # beta side is a cheap [128,81] vector recurrence after level 2
            # Pslab[k] = A_0..A_k (i,j)
            Pslab = dbig.tile([128, C64 * 81], F32, tag="Atmp")
            candf = dbig.tile([128, 729], F32, tag="candf")
            nc.vector.tensor_copy(out=Pslab[:, 0:81], in_=Aslab[:, 0:81])

            def m81(ap, kk):
                return ap[:, kk * 81:(kk + 1) * 81]

            def as_ij(ap3):
                return ap3.rearrange("p (i j) -> p i j", i=K)

            def jm(ap81):
                return ap81.rearrange("p (j m) -> p j m", j=K)

            for k in range(1, C64):
                pv = as_ij(m81(Pslab, k - 1)).unsqueeze(2).to_broadcast(
                    [128, K, K, K])                          # (i, j, m)
                av = jm(m81(ATslab, k)).unsqueeze(1).to_broadcast(
                    [128, K, K, K])                          # (i, j, m)
                nc.vector.tensor_tensor(
                    out=candf[:].rearrange("p (i j m) -> p i j m", i=K, j=K),
                    in0=pv, in1=av, op=Alu.add)
                nc.vector.tensor_reduce(
                    out=as_ij(m81(Pslab, k)),
                    in_=candf[:].rearrange("p (i j m) -> p i j m", i=K, j=K),
                    axis=AxX, op=Alu.max)

            # level 2: cross-chunk alpha-start / beta-end chains on p0-15
            Gg = dbig.tile([Bc, NC8 * 81], F32, tag="Gg")
            nc.sync.dma_start(out=Gg[:], in_=m81(Pslab, C64 - 1))
            ast = dbig.tile([Bc, NC8 * K], F32, tag="ast")
            bend = dbig.tile([Bc, NC8 * K], F32, tag="bend")
            candL = dbig.tile([Bc, 81], F32, tag="candL")
            nc.vector.tensor_tensor(
                out=ast[:, 0:K], in0=start9_sb[:], in1=em0_sb[:], op=Alu.add)
            for c in range(1, NC8):
                # ast_c[j] = max_i ast_{c-1}[i] + G_{c-1}[i,j]
                in0 = ast[:, (c - 1) * K:c * K].unsqueeze(1).to_broadcast(
                    [Bc, K, K])                               # (j, i)
                in1 = Gg[:, (c - 1) * 81:c * 81].rearrange(
                    "b (i j) -> b i j", i=K).transpose([0, 2, 1])
                nc.vector.tensor_tensor(
                    out=candL[:].rearrange("b (j i) -> b j i", j=K),
                    in0=in0, in1=in1, op=Alu.add)
                nc.vector.tensor_reduce(
                    out=ast[:, c * K:(c + 1) * K],
                    in_=candL[:].rearrange("b (j i) -> b j i", j=K),
                    axis=AxX, op=Alu.max)
            nc.vector.tensor_copy(
                out=bend[:, (NC8 - 1) * K:], in_=end9_sb[:])
            for c in range(NC8 - 2, -1, -1):
                # bend_c[i] = max_j G_{c+1}[i,j] + bend_{c+1}[j]
                in0 = bend[:, (c + 1) * K:(c + 2) * K].unsqueeze(1) \
                    .to_broadcast([Bc, K, K])                 # (i, j)
                in1 = Gg[:, (c + 1) * 81:(c + 2) * 81].rearrange(
                    "b (i j) -> b i j", i=K)
                nc.vector.tensor_tensor(
                    out=candL[:].rearrange("b (i j) -> b i j", i=K),
                    in0=in0, in1=in1, op=Alu.add)
                nc.vector.tensor_reduce(
                    out=bend[:, c * K:(c + 1) * K],
                    in_=candL[:].rearrange("b (i j) -> b i j", i=K),
                    axis=AxX, op=Alu.max)
            alpha_start = dbig.tile([128, K], F32, tag="astart")
            nc.sync.dma_start(out=alpha_start[:], in_=ast[:])
            beta_end = dbig.tile([128, K], F32, tag="bstart")
            nc.sync.dma_start(out=beta_end[:], in_=bend[:])

            # level 3: all per-step alpha/beta via one bulk op pair each
            cand3 = dbig.tile([128, C64 * 81], F32, tag="cand3")
            alpha_all = dbig.tile([128, C64 * K], F32, tag="em128")
            in0 = alpha_start[:].unsqueeze(1).unsqueeze(1).to_broadcast(
                [128, C64, K, K])                             # (k, j, i)
            in1 = Pslab[:].rearrange(
                "p (k i j) -> p k i j", k=C64, i=K).transpose([0, 1, 3, 2])
            nc.vector.tensor_tensor(
                out=cand3[:].rearrange("p (k j i) -> p k j i", k=C64, j=K),
                in0=in0, in1=in1, op=Alu.add)
            nc.vector.tensor_reduce(
                out=alpha_all[:].rearrange("p (k j) -> p k j", k=C64),
                in_=cand3[:].rearrange("p (k j i) -> p k j i", k=C64, j=K),
                axis=AxX, op=Alu.max)
            # beta vector recurrence: beta_k = A_{k+1} (x) beta_{k+1}
            beta_all = dbig.tile([128, C64 * K], F32, tag="ball")
            cand9 = dbig.tile([128, 81], F32, tag="cand9")
            nc.vector.tensor_copy(
                out=beta_all[:, (C64 - 1) * K:], in_=beta_end[:])
            for k in range(C64 - 2, -1, -1):
                in0 = as_ij(m81(Aslab, k + 1))                # (i, j)
                in1 = beta_all[:, (k + 1) * K:(k + 2) * K].unsqueeze(
                    1).to_broadcast([128, K, K])              # (i, j)
                nc.vector.tensor_tensor(
                    out=cand9[:].rearrange("p (i j) -> p i j", i=K),
                    in0=in0, in1=in1, op=Alu.add)
                nc.vector.tensor_reduce(
                    out=beta_all[:, k * K:(k + 1) * K],
                    in_=cand9[:].rearrange("p (i j) -> p i j", i=K),
                    axis=AxX, op=Alu.max)

            # ---------- P6: tags = argmax_i(alpha + beta), mask, emit ------
            nc.vector.tensor_tensor(
                out=alpha_all[:], in0=alpha_all[:], in1=beta_all[:],
                op=Alu.add)
            mx128 = dbig.tile([128, C64], F32, tag="mx128")
            nc.vector.tensor_reduce(
                out=mx128[:],
                in_=alpha_all[:].rearrange("p (k j) -> p k j", k=C64),
                axis=AxX, op=Alu.max)
            nc.vector.tensor_tensor(
                out=beta_all[:].rearrange("p (k j) -> p k j", k=C64),
                in0=alpha_all[:].rearrange("p (k j) -> p k j", k=C64),
                in1=mx128[:].unsqueeze(2).to_broadcast([128, C64, K]),
                op=Alu.is_equal)
            nc.vector.tensor_tensor(
                out=beta_all[:].rearrange("p (k j) -> p k j", k=C64),
                in0=beta_all[:].rearrange("p (k j) -> p k j", k=C64),
                in1=irev128_sb[:].unsqueeze(1).to_broadcast([128, C64, K]),
                op=Alu.mult)
            tags128 = dbig.tile([128, C64], F32, tag="tags128")
            nc.vector.tensor_reduce(
                out=tags128[:],
                in_=beta_all[:].rearrange("p (k j) -> p k j", k=C64),
                axis=AxX, op=Alu.max)
            nc.vector.tensor_scalar(
                out=tags128[:], in0=tags128[:], scalar1=-1.0, scalar2=8.0,
                op0=Alu.mult, op1=Alu.add)
            nc.vector.tensor_tensor(
                out=tags128[:], in0=tags128[:], in1=tagm_sb[:], op=Alu.mult)
            tagsi = dbig.tile([128, C64], I32, tag="tagsi")
            nc.vector.tensor_copy(out=tagsi[:], in_=tags128[:])
            nc.sync.dma_start(out=tags_out[:], in_=tagsi[:])
            dpbig_cm.__exit__(None, None, None)
            psp_cm.__exit__(None, None, None)

    nc.finalize()
    return nc


_NC_CACHE = None


def _get_program():
    global _NC_CACHE
    if _NC_CACHE is None:
        _NC_CACHE = build_program()
    return _NC_CACHE


def make_in_maps(sentences, lengths, embed, Wih_f, Whh_f, bih_f, bhh_f,
                 Wih_b, Whh_b, bih_b, bhh_b, W_out, b_out, start_t, end_t,
                 trans):
    sentences = np.ascontiguousarray(sentences, dtype=np.int32)
    embed = np.ascontiguousarray(embed, dtype=np.float32)
    lengths = np.asarray(lengths)

    # all-tanh pre-scaling: s = tanh(psum) must give tanh(x/2) for i,f,o
    # (gate blocks 0,1,3) and tanh(g) for block 2; h is stored as H=2h so
    # Whh additionally absorbs a 0.5.
    whh_pack = np.zeros((128, 1024), np.float32)
    wih_pack = np.zeros((128, 1024), np.float32)
    bias_pack = np.zeros((128, 8), np.float32)
    for d, (Wih, Whh, bi, bh) in enumerate(
            ((Wih_f, Whh_f, bih_f, bhh_f), (Wih_b, Whh_b, bih_b, bhh_b))):
        for g in range(G4):
            s_ih = 0.5 if g in (0, 1, 3) else 1.0
            s_hh = 0.25 if g in (0, 1, 3) else 0.5
            whh_pack[:, (d * G4 + g) * 128:(d * G4 + g + 1) * 128] = \
                np.asarray(Whh)[g * 128:(g + 1) * 128, :].T * s_hh
            wih_pack[:, (d * G4 + g) * 128:(d * G4 + g + 1) * 128] = \
                np.asarray(Wih)[g * 128:(g + 1) * 128, :].T * s_ih
            bias_pack[:, d * G4 + g] = \
                (np.asarray(bi) + np.asarray(bh))[g * 128:(g + 1) * 128] * s_ih

    W_out = np.asarray(W_out, np.float32) * 0.5  # H = 2h compensation
    woutT = np.zeros((128, 18), np.float32)
    woutT[:, 0:K] = W_out[:, :128].T
    woutT[:, K:2 * K] = W_out[:, 128:].T
    bout9 = np.asarray(b_out, np.float32)[:, None].copy()

    rev_id = np.zeros((128, 128), np.float32)
    rev_id[np.arange(128), 127 - np.arange(128)] = 1.0

    trans_np = np.asarray(trans, np.float32)
    identM = np.full((K, K), NEG, np.float32)
    np.fill_diagonal(identM, 0.0)
    trans81f = np.broadcast_to(trans_np.reshape(-1)[None], (128, 81)).copy()
    identM81f = np.broadcast_to(identM.reshape(-1)[None], (128, 81)).copy()

    start9 = np.broadcast_to(
        np.asarray(start_t, np.float32)[None, :], (Bc, K)).copy()
    end9 = np.broadcast_to(
        np.asarray(end_t, np.float32)[None, :], (Bc, K)).copy()
    ii = np.arange(K, dtype=np.float32)
    irev128 = np.broadcast_to((8.0 - ii)[None, :], (128, K)).copy()
    tt = np.arange(T)

    in_maps = []
    for c in range(NC):
        sl = slice(c * Bc, (c + 1) * Bc)
        sents_c = sentences[sl]
        lens_c = np.asarray(lengths[sl], np.float32)
        idx_np = np.zeros((128, NBLK), np.int32)
        p = np.arange(128)
        for g in range(NBLK):
            bt = g * 128 + p
            idx_np[:, g] = sents_c[bt // T, bt % T]
        mask_np = (tt[None, :] < lens_c[:, None]).astype(np.float32)
        # bwd i/f poison: s-index s corresponds to t = T-1-s; padded when
        # t >= len i.e. s < T - len
        ss = np.arange(T)
        mpois = np.zeros((Bc, T), np.float32)
        for b in range(Bc):
            mpois[b, ss < T - int(lens_c[b])] = POIS
        mpois128 = np.broadcast_to(
            mpois.reshape(1, -1), (128, TOK)).astype(np.float16).copy()
        # [128, 64] chunked masks, partition p = b*8 + c
        tagm = mask_np.reshape(Bc * 8, 64).copy()
        maskA_np = tagm.copy()
        maskA_np[0::8, 0] = 0.0  # A_0 := maxplus identity
        in_maps.append({
            "embed": embed,
            "idx": idx_np,
            "whh_pack": whh_pack.astype(np.float16),
            "wih_pack": wih_pack, "bias_pack": bias_pack,
            "mpois": mpois128,
            "woutT": woutT, "bout9": bout9, "rev_id": rev_id,
            "trans81f": trans81f, "identM81f": identM81f,
            "maskA": maskA_np, "invmA": 1.0 - maskA_np, "tagm128": tagm,
            "irev128": irev128,
            "start9": start9, "end9": end9,
        })
    return in_maps


def run(inputs, trace=False, **kw):
    nc = _get_program()
    in_maps = make_in_maps(**inputs)
    res = run_bass_kernel_spmd(nc, in_maps, list(range(NC)), trace=trace, **kw)
    tags = np.concatenate([r["tags"] for r in res.results], axis=0)
    return tags.astype(np.int32), res


def kernel(**inputs):
    tags, _ = run(inputs)
    return tags


# revision 20
# speedup vs baseline: 1.3470x; 1.0001x over previous
"""BiLSTM-CRF Viterbi decode on 8 Trainium2 NeuronCores.

Data-parallel over batch: each core handles 16 of 128 sequences.

Per-core phases:
  P0 embedding gather (indirect DMA, 128 rows per DMA)
  P1 PE-transpose x_rows [tok,E] -> x_T [E,tok] and x_Trev (time-reversed
     per sequence, via anti-diagonal identity)
  P2 bulk input projection xproj = Wih_g @ x_T (+bias) staged to DRAM fp16,
     fp32r matmuls (N=512); bwd direction projected from x_Trev so its
     DRAM layout is s-ordered (s = T-1-t). Bwd i/f gate lanes at padded
     steps are poisoned to -1e4 so tanh saturates to -1 and the cell
     state provably stays zero through the padded prefix (replaces all
     per-round masking).
  P3 512 fused fwd+bwd LSTM rounds, all-tanh formulation:
     host pre-scales weights so ONE tanh over the [128,128] gate PSUM
     yields s = tanh(x/2) for i,f,o (i.e. 2*sigmoid-1) and tanh(g) for g.
     Cell state kept as C = 2c, hidden as H = 2h:
        t1 = (s_i+1)*tg          (= 2 sig_i tg)
        t2 = (s_f+1)*C           (= 2 sig_f C)
        C' = t2*0.5 + t1         (= 2c')
        tcx = tanh(0.5*C')       (= tanh(c'))
        H' = (s_o+1)*tcx         (= 2h')
     xproj joins the gates via an fp16 identity matmul accumulating into
     the same PSUM bank (no DVE add). Both directions' H live in one
     [128,32] slot per round (bwd s-ordered) so one STT writes both and
     all 8 recurrence matmuls wait on a single semaphore.
  P4 emissions em^T = W_out @ h as [9,512] fp16 matmuls (strided /
     reverse-strided rhs straight out of h_all) staged to DRAM
  P5 blocked max-plus scan Viterbi: time-chunks on partitions
     (p = b*8 + c, 64 steps/chunk), within-chunk prefix/suffix 9x9
     max-plus matrix products on DVE, tiny cross-chunk chains, then one
     bulk op pair recovers every alpha_t / beta_t
  P6 tags_t = argmax_i(alpha_t + beta_t), bulk [128, 64*9] ops

Precision: xproj fp32r matmul staged fp16, recurrence fp16, DP fp32.
"""

import ml_dtypes
import numpy as np

import concourse.bacc as bacc
import concourse.bass as bass
import concourse.mybir as mybir
import concourse.tile as tile
from concourse.bass import IndirectOffsetOnAxis
from concourse.bass_utils import run_bass_kernel_spmd
from concourse.masks import make_identity

F32 = mybir.dt.float32
F32R = mybir.dt.float32r
BF16 = mybir.dt.bfloat16
F16 = mybir.dt.float16
I32 = mybir.dt.int32
Alu = mybir.AluOpType
Act = mybir.ActivationFunctionType
AxX = mybir.AxisListType.X

B, T, V, E, H, K = 128, 512, 100000, 128, 128, 9
NC = 8
Bc = B // NC          # 16 sequences per core
TOK = Bc * T          # 8192 tokens per core, flat index bt = b*T + t (b-major)
NBLK = TOK // 128     # 64 gather/transpose blocks
G4 = 4
# gate order in weights: i, f, g, o (torch). psum cols per dir: i(0) f(16) o(32) g(48)
GOFF = {0: 0, 1: 16, 3: 32, 2: 48}
CH = 32               # LSTM rounds per xproj chunk
NCH = T // CH
NEG = -1.0e9
POIS = -1.0e4         # bwd i/f gate poison at padded steps
XDT = F16             # xproj staging dtype


def f32(ap):
    return ap.bitcast(F32)


def build_program():
    nc = bacc.Bacc(None, target_bir_lowering=False)

    # ---------------- dram parameters ----------------
    embed = nc.declare_dram_parameter("embed", [V, E], F32, isOutput=False)
    idx = nc.declare_dram_parameter("idx", [128, NBLK], I32, isOutput=False)
    whh_pack = nc.declare_dram_parameter("whh_pack", [128, 1024], F16, isOutput=False)
    wih_pack = nc.declare_dram_parameter("wih_pack", [128, 1024], F32, isOutput=False)
    bias_pack = nc.declare_dram_parameter("bias_pack", [128, 8], F32, isOutput=False)
    mpois = nc.declare_dram_parameter("mpois", [128, TOK], F16, isOutput=False)
    woutT = nc.declare_dram_parameter("woutT", [128, 18], F32, isOutput=False)
    bout9 = nc.declare_dram_parameter("bout9", [K, 1], F32, isOutput=False)
    rev_id = nc.declare_dram_parameter("rev_id", [128, 128], F32, isOutput=False)
    trans81f = nc.declare_dram_parameter("trans81f", [128, 81], F32, isOutput=False)
    identM81f = nc.declare_dram_parameter("identM81f", [128, 81], F32, isOutput=False)
    maskA = nc.declare_dram_parameter("maskA", [128, 64], F32, isOutput=False)
    invmA = nc.declare_dram_parameter("invmA", [128, 64], F32, isOutput=False)
    tagm128 = nc.declare_dram_parameter("tagm128", [128, 64], F32, isOutput=False)
    irev128 = nc.declare_dram_parameter("irev128", [128, K], F32, isOutput=False)
    start9 = nc.declare_dram_parameter("start9", [Bc, K], F32, isOutput=False)
    end9 = nc.declare_dram_parameter("end9", [Bc, K], F32, isOutput=False)
    tags_out = nc.declare_dram_parameter("tags", [Bc, T], I32, isOutput=True)

    # ---------------- dram internals ----------------
    xproj_dram = nc.dram_tensor("xproj_dram", [2, G4, Bc, 128, T], XDT)
    emT_dram = nc.dram_tensor("emT_dram", [K, TOK], F32)

    with tile.TileContext(nc) as tc:
        with (
            tc.tile_pool(name="big", bufs=1) as big,
            tc.tile_pool(name="consts", bufs=1) as cst,
            tc.tile_pool(name="small", bufs=4) as sm,
        ):
            # ---------- constants ----------
            idx_sb = cst.tile([128, NBLK], I32)
            nc.sync.dma_start(out=idx_sb[:], in_=idx[:])
            whh_sb = cst.tile([128, 1024], F16)
            nc.sync.dma_start(out=whh_sb[:], in_=whh_pack[:])
            wih_sb = cst.tile([128, 1024], F32)
            nc.sync.dma_start(out=wih_sb[:], in_=wih_pack[:])
            bias_sb = cst.tile([128, 8], F32)
            nc.sync.dma_start(out=bias_sb[:], in_=bias_pack[:])
            mpp_cm = tc.tile_pool(name="mpp", bufs=1)
            mpp = mpp_cm.__enter__()
            mpois_sb = mpp.tile([128, TOK], F16)
            nc.sync.dma_start(out=mpois_sb[:], in_=mpois[:])
            woutT_sb = cst.tile([128, 18], F32)
            nc.sync.dma_start(out=woutT_sb[:], in_=woutT[:])
            # device-rounded fp32r copies (the fp32r matmul path requires
            # its operands to be produced pre-rounded to fp32r)
            wihr_sb = cst.tile([128, 1024], F32R)
            nc.vector.tensor_copy(out=wihr_sb[:], in_=wih_sb[:])
            woutTr_sb = cst.tile([128, 18], F16)
            nc.vector.tensor_copy(out=woutTr_sb[:], in_=woutT_sb[:])
            bout9_sb = cst.tile([K, 1], F32)
            nc.sync.dma_start(out=bout9_sb[:], in_=bout9[:])
            rev_dma = cst.tile([128, 128], F32)
            nc.sync.dma_start(out=rev_dma[:], in_=rev_id[:])
            rev_sb = cst.tile([128, 128], F32)
            nc.vector.tensor_copy(out=rev_sb[:], in_=rev_dma[:])
            ident = cst.tile([128, 128], F32)
            make_identity(nc, ident[:])
            ident16 = cst.tile([128, 128], F16)
            nc.vector.tensor_copy(out=ident16[:], in_=ident[:])

            # DP constants (p0-15)
            start9_sb = cst.tile([Bc, K], F32)
            nc.sync.dma_start(out=start9_sb[:], in_=start9[:])
            end9_sb = cst.tile([Bc, K], F32)
            nc.sync.dma_start(out=end9_sb[:], in_=end9[:])

            # PE "absorber" ops: self-loading (fp32/fp32r) matmuls may carry
            # at most ONE sync wait in walrus codegen. These tiny ops advance
            # PE's vector clock over one-time deps (identity from Pool,
            # const-weight DMA lanes) so real matmuls each need <=1 wait.
            psp_cm = tc.tile_pool(name="psglob", bufs=1, space="PSUM")
            psp = psp_cm.__enter__()
            pq1 = psp.tile([128, 512], F32, tag="pq1", name="pq1")
            pq2 = psp.tile([128, 512], F32, tag="pq2", name="pq2")
            pw1 = psp.tile([128, 512], F32, tag="pw1", name="pw1")
            pw2 = psp.tile([128, 512], F32, tag="pw2", name="pw2")
            pw3 = psp.tile([128, 512], F32, tag="pw3", name="pw3")
            nc.tensor.transpose(out=pq1[:, 0:128], in_=ident[:], identity=ident[:])
            nc.tensor.transpose(out=pq2[:, 0:128], in_=rev_sb[:], identity=ident[:])
            nc.tensor.matmul(out=pq2[0:1, 0:1], lhsT=whh_sb[:, 0:1],
                             rhs=whh_sb[:, 0:1], start=True, stop=True)
            nc.tensor.matmul(out=pq1[0:1, 0:1], lhsT=ident16[:, 0:1],
                             rhs=whh_sb[:, 0:1], start=True, stop=True)

            # ---------- P0: gather ----------
            x_T = big.tile([128, TOK], F32R, tag="bigA")
            with tc.tile_pool(name="xr", bufs=24) as xrp:
                x_rows = []
                for g in range(NBLK):
                    xr = xrp.tile([128, 128], F32, tag="xr")
                    nc.gpsimd.indirect_dma_start(
                        out=xr[:],
                        out_offset=None,
                        in_=embed[:],
                        in_offset=IndirectOffsetOnAxis(
                            ap=idx_sb[:, g:g + 1], axis=0),
                    )
                    x_rows.append(xr)

                # ---------- P1: transpose (fwd only; bwd read reversed) ----
                with tc.tile_pool(name="xrel", bufs=4) as xrelp:
                    psts = [pq1, pq2]
                    for g in range(NBLK):
                        xrel = xrelp.tile([128, 128], F32, tag="xrel")
                        nc.vector.tensor_tensor(
                            out=xrel[:], in0=x_rows[g][:], in1=x_rows[g][:],
                            op=Alu.max)
                        pst = psts[g % 2]
                        nc.tensor.transpose(
                            out=pst[:, 0:128], in_=xrel[:], identity=ident[:])
                        if g % 2 == 0:
                            nc.vector.tensor_copy(
                                out=x_T[:, g * 128:(g + 1) * 128],
                                in_=pst[:, 0:128])
                        else:
                            nc.scalar.activation(
                                out=x_T[:, g * 128:(g + 1) * 128],
                                in_=pst[:, 0:128], func=Act.Copy)

            # ---------- P2: bulk xproj (fp32r, N=512) ----------
            # bwd (d=1) reads x_T with reversed t so its DRAM layout is
            # s-ordered without needing a second transposed copy
            xv = x_T[:].rearrange("p (b t) -> p b t", b=Bc)
            ps2s = [pw1[:], pw2[:], pw3[:]]
            n2 = 0
            # b outermost: block (d,g,b) only needs sequence b's gathers, so
            # P2 pipelines behind P0/P1 instead of stalling on the last one
            for b in range(Bc):
                for d in range(2):
                    for g in range(G4):
                        lhsT = wihr_sb[
                            :, (d * G4 + g) * 128:(d * G4 + g + 1) * 128]
                        ps2 = ps2s[n2 % 3]
                        n2 += 1
                        rhs = xv[:, b] if d == 0 else xv[:, b, ::-1]
                        nc.tensor.matmul(
                            out=ps2, lhsT=lhsT, rhs=rhs,
                            start=True, stop=True)
                        xp_sb = sm.tile([128, 512], XDT, tag="xp_out")
                        if d == 1 and g in (0, 1):
                            # poison bwd i/f gate lanes at padded steps
                            nc.vector.scalar_tensor_tensor(
                                out=xp_sb[:], in0=ps2,
                                scalar=bias_sb[:, d * G4 + g:d * G4 + g + 1],
                                in1=mpois_sb[:, b * T:(b + 1) * T],
                                op0=Alu.add, op1=Alu.add)
                        elif n2 % 2 == 0:
                            nc.vector.tensor_scalar(
                                out=xp_sb[:], in0=ps2,
                                scalar1=bias_sb[:, d * G4 + g:d * G4 + g + 1],
                                scalar2=None, op0=Alu.add)
                        else:
                            nc.scalar.activation(
                                out=xp_sb[:], in_=ps2, func=Act.Identity,
                                bias=bias_sb[:, d * G4 + g:d * G4 + g + 1])
                        # store at PSUM block position (i,f,o,g order)
                        nc.sync.dma_start(
                            out=xproj_dram[d, GOFF[g] // 16, b],
                            in_=xp_sb[:])

            mpp_cm.__exit__(None, None, None)

            # ---------- P3: LSTM (all-tanh, C=2c / H=2h) ----------
            # h_all: slot r holds [H_f(t=r) | H_b(s=r)] fp16
            h_all = big.tile([128, T * 32], F16, tag="bigA")
            h0 = cst.tile([128, 32], F16)
            nc.vector.memset(h0[:], 0.0)
            c_st = cst.tile([128, 2 * Bc], F32)
            nc.vector.memset(c_st[:], 0.0)

            with tc.tile_pool(name="xpp", bufs=2) as xpp:
                for r in range(T):
                    c = r // CH
                    if r % CH == 0:
                        # layout [128, (y=(d, gslot, b), s)]: matches psum
                        # column order (d,g,b) when sliced at fixed s
                        xt = xpp.tile([128, CH * 128], XDT, tag="xpc")
                        for d in range(2):
                            dst = xt[:].rearrange(
                                "p (y s) -> p y s",
                                s=CH)[:, d * 64:(d + 1) * 64]
                            src = xproj_dram[:].rearrange(
                                "e g b p t -> p (e g b) t")[
                                :, d * 64:(d + 1) * 64,
                                c * CH:(c + 1) * CH]
                            nc.sync.dma_start(out=dst, in_=src)

                    ps3 = [pq1, pq2, pw3][r % 3]
                    # xproj into PSUM first (no h dependency), gates accum
                    xsl = xt[:].rearrange(
                        "p (y s) -> p y s", s=CH)[:, :, r % CH]
                    nc.tensor.matmul(
                        out=ps3[:, 0:128], lhsT=ident16[:], rhs=xsl,
                        start=True, stop=False)
                    hprev = h0[:] if r == 0 else h_all[:, (r - 1) * 32:r * 32]
                    for d in range(2):
                        for g in range(G4):
                            lhsT = whh_sb[
                                :, (d * G4 + g) * 128:(d * G4 + g + 1) * 128]
                            nc.tensor.matmul(
                                out=ps3[:, d * 64 + GOFF[g]:
                                        d * 64 + GOFF[g] + Bc],
                                lhsT=lhsT,
                                rhs=hprev[:, d * Bc:(d + 1) * Bc],
                                start=False, stop=(d == 1 and g == 3),
                                skip_group_check=True)

                    s_sb = sm.tile([128, 128], F32, tag="s_sb")
                    nc.scalar.activation(
                        out=s_sb[:], in_=ps3[:, 0:128], func=Act.Tanh)
                    s3 = s_sb[:].rearrange("p (d x) -> p d x", d=2)
                    t1 = sm.tile([128, 2 * Bc], F32, tag="t1")
                    nc.vector.scalar_tensor_tensor(
                        out=t1[:].rearrange("p (d x) -> p d x", d=2),
                        in0=s3[:, :, 0:16], scalar=1.0,
                        in1=s3[:, :, 48:64], op0=Alu.add, op1=Alu.mult)
                    t2 = sm.tile([128, 2 * Bc], F32, tag="t2")
                    nc.vector.scalar_tensor_tensor(
                        out=t2[:].rearrange("p (d x) -> p d x", d=2),
                        in0=s3[:, :, 16:32], scalar=1.0,
                        in1=c_st[:].rearrange("p (d x) -> p d x", d=2),
                        op0=Alu.add, op1=Alu.mult)
                    nc.vector.scalar_tensor_tensor(
                        out=c_st[:], in0=t2[:], scalar=0.5, in1=t1[:],
                        op0=Alu.mult, op1=Alu.add)
                    tcx = sm.tile([128, 2 * Bc], F32, tag="tc")
                    nc.scalar.activation(
                        out=tcx[:], in_=c_st[:], func=Act.Tanh, scale=0.5)
                    nc.vector.scalar_tensor_tensor(
                        out=h_all[:, r * 32:(r + 1) * 32].rearrange(
                            "p (d x) -> p d x", d=2),
                        in0=s3[:, :, 32:48], scalar=1.0,
                        in1=tcx[:].rearrange("p (d x) -> p d x", d=2),
                        op0=Alu.add, op1=Alu.mult)

            # ---------- P4: emissions em^T = [9, TOK] (fp16, N=512) ----------
            for blk in range(Bc):
                ps4 = [pw1, pw2][blk % 2][0:K, :]
                sl = slice(blk * T, (blk + 1) * T)
                hf_ap = h_all[:, blk::32]
                hb_ap = h_all[:, (T - 1) * 32 + 16 + blk:blk:-32]
                nc.tensor.matmul(
                    out=ps4, lhsT=woutTr_sb[:, 0:K], rhs=hf_ap,
                    start=True, stop=False)
                nc.tensor.matmul(
                    out=ps4, lhsT=woutTr_sb[:, K:2 * K], rhs=hb_ap,
                    start=False, stop=True)
                em_sb = sm.tile([K, T], F32, tag="em_sb")
                nc.vector.tensor_scalar(
                    out=em_sb[:], in0=ps4, scalar1=bout9_sb[:, 0:1],
                    scalar2=None, op0=Alu.add)
                nc.sync.dma_start(out=emT_dram[:, sl], in_=em_sb[:])

            # ---------- P5: blocked max-plus scan Viterbi ----------
            # partition p = b*8 + c: sequence b, time-chunk c (C=64 steps).
            # A_t[i,j] = mask_t ? trans[i,j] + em_t[j] : maxplus identity
            # (0 diag / -1e9 off); A_0 := I via maskA[.,0]=0 host fudge.
            # alpha_t = alpha_{t-1} (x) A_t  (row-vec max-plus)
            # beta_t  = A_{t+1} (x) beta_{t+1}  (col-vec max-plus)
            # tags_t  = argmax_i(alpha_t[i] + beta_t[i])
            C64, NC8 = 64, 8
            dpbig_cm = tc.tile_pool(name="dpbig", bufs=1)
            dbig = dpbig_cm.__enter__()

            em128 = dbig.tile([128, K * C64], F32, tag="em128")  # (j, k)
            nc.sync.dma_start(
                out=em128[:].rearrange("p (j k) -> p j k", j=K),
                in_=emT_dram[:].rearrange(
                    "k (b c t) -> (b c) k t", b=Bc, c=NC8))
            em0_sb = dbig.tile([Bc, K], F32, tag="em0")
            nc.sync.dma_start(
                out=em0_sb[:].unsqueeze(2),
                in_=emT_dram[:].rearrange("k (b t) -> b k t", b=Bc)[:, :, 0:1])

            trans128_sb = dbig.tile([128, 81], F32, tag="t128")
            nc.sync.dma_start(out=trans128_sb[:], in_=trans81f[:])
            identM128_sb = dbig.tile([128, 81], F32, tag="i128")
            nc.sync.dma_start(out=identM128_sb[:], in_=identM81f[:])
            maskA_sb = dbig.tile([128, C64], F32, tag="mA")
            nc.sync.dma_start(out=maskA_sb[:], in_=maskA[:])
            invmA_sb = dbig.tile([128, C64], F32, tag="imA")
            nc.sync.dma_start(out=invmA_sb[:], in_=invmA[:])
            tagm_sb = dbig.tile([128, C64], F32, tag="tagm")
            nc.sync.dma_start(out=tagm_sb[:], in_=tagm128[:])
            irev128_sb = dbig.tile([128, K], F32, tag="irev128")
            nc.sync.dma_start(out=irev128_sb[:], in_=irev128[:])

            # A slab build: A = m*(trans+em) + invm*I  (exact-zero masking)
            Aslab = dbig.tile([128, C64 * 81], F32, tag="Aslab")
            Atmp = dbig.tile([128, C64 * 81], F32, tag="Atmp")

            def A4(t_):
                return t_[:].rearrange("p (s i j) -> p s i j", s=C64, i=K)

            emv = em128[:].rearrange("p (j k) -> p k j", j=K)
            emv = emv.unsqueeze(2).to_broadcast([128, C64, K, K])
            t128v = trans128_sb[:].rearrange("p (i j) -> p i j", i=K)
            t128v = t128v.unsqueeze(1).to_broadcast([128, C64, K, K])
            i128v = identM128_sb[:].rearrange("p (i j) -> p i j", i=K)
            i128v = i128v.unsqueeze(1).to_broadcast([128, C64, K, K])
            mAv = maskA_sb[:].bitcast(I32).unsqueeze(2).unsqueeze(3) \
                .to_broadcast([128, C64, K, K])
            imAv = invmA_sb[:].unsqueeze(2).unsqueeze(3).to_broadcast(
                [128, C64, K, K])
            nc.vector.tensor_tensor(out=A4(Atmp), in0=emv, in1=t128v,
                                    op=Alu.add)
            nc.vector.tensor_copy(out=A4(Aslab), in_=i128v)
            nc.vector.copy_predicated(out=A4(Aslab), mask=mAv, data=A4(Atmp))

            # transposed A copy: ATslab[s][(j,m)] = A_s[m,j] — makes every
            # level-1 operand innermost-contiguous (strided adds are ~1.7x)
            ATslab = big.tile([128, C64 * 81], F32, tag="bigB")
            nc.vector.tensor_copy(
                out=ATslab[:].rearrange("p (s j m) -> p s j m", s=C64, j=K),
                in_=A4(Aslab).transpose([0, 1, 3, 2]))

            # level 1: within-chunk prefix products only; the per-step
            # beta side is a cheap [128,81] vector recurrence after level 2
            # Pslab[k] = A_0..A_k (i,j)
            Pslab = dbig.tile([128, C64 * 81], F32, tag="Atmp")
            candf = dbig.tile([128, 729], F32, tag="candf")
            nc.vector.tensor_copy(out=Pslab[:, 0:81], in_=Aslab[:, 0:81])

            def m81(ap, kk):
                return ap[:, kk * 81:(kk + 1) * 81]

            def as_ij(ap3):
                return ap3.rearrange("p (i j) -> p i j", i=K)

            def jm(ap81):
                return ap81.rearrange("p (j m) -> p j m", j=K)

            for k in range(1, C64):
                pv = as_ij(m81(Pslab, k - 1)).unsqueeze(2).to_broadcast(
                    [128, K, K, K])                          # (i, j, m)
                av = jm(m81(ATslab, k)).unsqueeze(1).to_broadcast(
                    [128, K, K, K])                          # (i, j, m)
                nc.vector.tensor_tensor(
                    out=candf[:].rearrange("p (i j m) -> p i j m", i=K, j=K),
                    in0=pv, in1=av, op=Alu.add)
                nc.vector.tensor_reduce(
                    out=as_ij(m81(Pslab, k)),
                    in_=candf[:].rearrange("p (i j m) -> p i j m", i=K, j=K),
                    axis=AxX, op=Alu.max)

            # level 2: cross-chunk alpha-start / beta-end chains on p0-15
            Gg = dbig.tile([Bc, NC8 * 81], F32, tag="Gg")
            nc.sync.dma_start(out=Gg[:], in_=m81(Pslab, C64 - 1))
            ast = dbig.tile([Bc, NC8 * K], F32, tag="ast")
            bend = dbig.tile([Bc, NC8 * K], F32, tag="bend")
            candL = dbig.tile([Bc, 81], F32, tag="candL")
            nc.vector.tensor_tensor(
                out=ast[:, 0:K], in0=start9_sb[:], in1=em0_sb[:], op=Alu.add)
            for c in range(1, NC8):
                # ast_c[j] = max_i ast_{c-1}[i] + G_{c-1}[i,j]
                in0 = ast[:, (c - 1) * K:c * K].unsqueeze(1).to_broadcast(
                    [Bc, K, K])                               # (j, i)
                in1 = Gg[:, (c - 1) * 81:c * 81].rearrange(
                    "b (i j) -> b i j", i=K).transpose([0, 2, 1])
                nc.vector.tensor_tensor(
                    out=candL[:].rearrange("b (j i) -> b j i", j=K),
                    in0=in0, in1=in1, op=Alu.add)
                nc.vector.tensor_reduce(
                    out=ast[:, c * K:(c + 1) * K],
                    in_=candL[:].rearrange("b (j i) -> b j i", j=K),
                    axis=AxX, op=Alu.max)
            nc.vector.tensor_copy(
                out=bend[:, (NC8 - 1) * K:], in_=end9_sb[:])
            for c in range(NC8 - 2, -1, -1):
                # bend_c[i] = max_j G_{c+1}[i,j] + bend_{c+1}[j]
                in0 = bend[:, (c + 1) * K:(c + 2) * K].unsqueeze(1) \
                    .to_broadcast([Bc, K, K])                 # (i, j)
                in1 = Gg[:, (c + 1) * 81:(c + 2) * 81].rearrange(
                    "b (i j) -> b i j", i=K)
                nc.vector.tensor_tensor(
                    out=candL[:].rearrange("b (i j) -> b i j", i=K),
                    in0=in0, in1=in1, op=Alu.add)
                nc.vector.tensor_reduce(
                    out=bend[:, c * K:(c + 1) * K],
                    in_=candL[:].rearrange("b (i j) -> b i j", i=K),
                    axis=AxX, op=Alu.max)
            alpha_start = dbig.tile([128, K], F32, tag="astart")
            nc.sync.dma_start(out=alpha_start[:], in_=ast[:])
            beta_end = dbig.tile([128, K], F32, tag="bstart")
            nc.sync.dma_start(out=beta_end[:], in_=bend[:])

            # level 3: all per-step alpha/beta via one bulk op pair each
            cand3 = dbig.tile([128, C64 * 81], F32, tag="cand3")
            alpha_all = dbig.tile([128, C64 * K], F32, tag="em128")
            in0 = alpha_start[:].unsqueeze(1).unsqueeze(1).to_broadcast(
                [128, C64, K, K])                             # (k, j, i)
            in1 = Pslab[:].rearrange(
                "p (k i j) -> p k i j", k=C64, i=K).transpose([0, 1, 3, 2])
            nc.vector.tensor_tensor(
                out=cand3[:].rearrange("p (k j i) -> p k j i", k=C64, j=K),
                in0=in0, in1=in1, op=Alu.add)
            nc.vector.tensor_reduce(
                out=alpha_all[:].rearrange("p (k j) -> p k j", k=C64),
                in_=cand3[:].rearrange("p (k j i) -> p k j i", k=C64, j=K),
                axis=AxX, op=Alu.max)
            # beta vector recurrence: beta_k = A_{k+1} (x) beta_{k+1}
            beta_all = dbig.tile([128, C64 * K], F32, tag="ball")
            cand9 = dbig.tile([128, 81], F32, tag="cand9")
            nc.vector.tensor_copy(
                out=beta_all[:, (C64 - 1) * K:], in_=beta_end[:])
            for k in range(C64 - 2, -1, -1):
                in0 = as_ij(m81(Aslab, k + 1))                # (i, j)
                in1 = beta_all[:, (k + 1) * K:(k + 2) * K].unsqueeze(
                    1).to_broadcast([128, K, K])              # (i, j)
                nc.vector.tensor_tensor(
                    out=cand9[:].rearrange("p (i j) -> p i j", i=K),
                    in0=in0, in1=in1, op=Alu.add)
                nc.vector.tensor_reduce(
                    out=beta_all[:, k * K:(k + 1) * K],
                    in_=cand9[:].rearrange("p (i j) -> p i j", i=K),
                    axis=AxX, op=Alu.max)

            # ---------- P6: tags = argmax_i(alpha + beta), mask, emit ------
            nc.vector.tensor_tensor(
                out=alpha_all[:], in0=alpha_all[:], in1=beta_all[:],
                op=Alu.add)
            mx128 = dbig.tile([128, C64], F32, tag="mx128")
            nc.vector.tensor_reduce(
                out=mx128[:],
                in_=alpha_all[:].rearrange("p (k j) -> p k j", k=C64),
                axis=AxX, op=Alu.max)
            nc.vector.tensor_tensor(
                out=beta_all[:].rearrange("p (k j) -> p k j", k=C64),
                in0=alpha_all[:].rearrange("p (k j) -> p k j", k=C64),
                in1=mx128[:].unsqueeze(2).to_broadcast([128, C64, K]),
                op=Alu.is_equal)
            nc.vector.tensor_tensor(
                out=beta_all[:].rearrange("p (k j) -> p k j", k=C64),
                in0=beta_all[:].rearrange("p (k j) -> p k j", k=C64),
                in1=irev128_sb[:].unsqueeze(1).to_broadcast([128, C64, K]),
                op=Alu.mult)
            tags128 = dbig.tile([128, C64], F32, tag="tags128")
            nc.vector.tensor_reduce(
                out=tags128[:],
                in_=beta_all[:].rearrange("p (k j) -> p k j", k=C64),
                axis=AxX, op=Alu.max)
            nc.vector.tensor_scalar(
                out=tags128[:], in0=tags128[:], scalar1=-1.0, scalar2=8.0,
                op0=Alu.mult, op1=Alu.add)
            nc.vector.tensor_tensor(
                out=tags128[:], in0=tags128[:], in1=tagm_sb[:], op=Alu.mult)
            tagsi = dbig.tile([128, C64], I32, tag="tagsi")
            nc.vector.tensor_copy(out=tagsi[:], in_=tags128[:])
            nc.sync.dma_start(out=tags_out[:], in_=tagsi[:])
            dpbig_cm.__exit__(None, None, None)
            psp_cm.__exit__(None, None, None)

    nc.finalize()
    return nc


_NC_CACHE = None


def _get_program():
    global _NC_CACHE
    if _NC_CACHE is None:
        _NC_CACHE = build_program()
    return _NC_CACHE


def make_in_maps(sentences, lengths, embed, Wih_f, Whh_f, bih_f, bhh_f,
                 Wih_b, Whh_b, bih_b, bhh_b, W_out, b_out, start_t, end_t,
                 trans):
    sentences = np.ascontiguousarray(sentences, dtype=np.int32)
    embed = np.ascontiguousarray(embed, dtype=np.float32)
    lengths = np.asarray(lengths)

    # all-tanh pre-scaling: s = tanh(psum) must give tanh(x/2) for i,f,o
    # (gate blocks 0,1,3) and tanh(g) for block 2; h is stored as H=2h so
    # Whh additionally absorbs a 0.5.
    whh_pack = np.zeros((128, 1024), np.float32)
    wih_pack = np.zeros((128, 1024), np.float32)
    bias_pack = np.zeros((128, 8), np.float32)
    for d, (Wih, Whh, bi, bh) in enumerate(
            ((Wih_f, Whh_f, bih_f, bhh_f), (Wih_b, Whh_b, bih_b, bhh_b))):
        for g in range(G4):
            s_ih = 0.5 if g in (0, 1, 3) else 1.0
            s_hh = 0.25 if g in (0, 1, 3) else 0.5
            whh_pack[:, (d * G4 + g) * 128:(d * G4 + g + 1) * 128] = \
                np.asarray(Whh)[g * 128:(g + 1) * 128, :].T * s_hh
            wih_pack[:, (d * G4 + g) * 128:(d * G4 + g + 1) * 128] = \
                np.asarray(Wih)[g * 128:(g + 1) * 128, :].T * s_ih
            bias_pack[:, d * G4 + g] = \
                (np.asarray(bi) + np.asarray(bh))[g * 128:(g + 1) * 128] * s_ih

    W_out = np.asarray(W_out, np.float32) * 0.5  # H = 2h compensation
    woutT = np.zeros((128, 18), np.float32)
    woutT[:, 0:K] = W_out[:, :128].T
    woutT[:, K:2 * K] = W_out[:, 128:].T
    bout9 = np.asarray(b_out, np.float32)[:, None].copy()

    rev_id = np.zeros((128, 128), np.float32)
    rev_id[np.arange(128), 127 - np.arange(128)] = 1.0

    trans_np = np.asarray(trans, np.float32)
    identM = np.full((K, K), NEG, np.float32)
    np.fill_diagonal(identM, 0.0)
    trans81f = np.broadcast_to(trans_np.reshape(-1)[None], (128, 81)).copy()
    identM81f = np.broadcast_to(identM.reshape(-1)[None], (128, 81)).copy()

    start9 = np.broadcast_to(
        np.asarray(start_t, np.float32)[None, :], (Bc, K)).copy()
    end9 = np.broadcast_to(
        np.asarray(end_t, np.float32)[None, :], (Bc, K)).copy()
    ii = np.arange(K, dtype=np.float32)
    irev128 = np.broadcast_to((8.0 - ii)[None, :], (128, K)).copy()
    tt = np.arange(T)

    in_maps = []
    for c in range(NC):
        sl = slice(c * Bc, (c + 1) * Bc)
        sents_c = sentences[sl]
        lens_c = np.asarray(lengths[sl], np.float32)
        idx_np = np.zeros((128, NBLK), np.int32)
        p = np.arange(128)
        for g in range(NBLK):
            bt = g * 128 + p
            idx_np[:, g] = sents_c[bt // T, bt % T]
        mask_np = (tt[None, :] < lens_c[:, None]).astype(np.float32)
        # bwd i/f poison: s-index s corresponds to t = T-1-s; padded when
        # t >= len i.e. s < T - len
        ss = np.arange(T)
        mpois = np.zeros((Bc, T), np.float32)
        for b in range(Bc):
            mpois[b, ss < T - int(lens_c[b])] = POIS
        mpois128 = np.broadcast_to(
            mpois.reshape(1, -1), (128, TOK)).astype(np.float16).copy()
        # [128, 64] chunked masks, partition p = b*8 + c
        tagm = mask_np.reshape(Bc * 8, 64).copy()
        maskA_np = tagm.copy()
        maskA_np[0::8, 0] = 0.0  # A_0 := maxplus identity
        in_maps.append({
            "embed": embed,
            "idx": idx_np,
            "whh_pack": whh_pack.astype(np.float16),
            "wih_pack": wih_pack, "bias_pack": bias_pack,
            "mpois": mpois128,
            "woutT": woutT, "bout9": bout9, "rev_id": rev_id,
            "trans81f": trans81f, "identM81f": identM81f,
            "maskA": maskA_np, "invmA": 1.0 - maskA_np, "tagm128": tagm,
            "irev128": irev128,
            "start9": start9, "end9": end9,
        })
    return in_maps


def run(inputs, trace=False, **kw):
    nc = _get_program()
    in_maps = make_in_maps(**inputs)
    res = run_bass_kernel_spmd(nc, in_maps, list(range(NC)), trace=trace, **kw)
    tags = np.concatenate([r["tags"] for r in res.results], axis=0)
    return tags.astype(np.int32), res


def kernel(**inputs):
    tags, _ = run(inputs)
    return tags
